# revision 17
# baseline (speedup 1.0000x reference)
"""AnchorAttention Trainium2 kernel (8 NeuronCores, SPMD, no collectives).

Math (per batch): gather anchor rows of hidden_states, LayerNorm, QKV
projections, dense attention among anchors only, out-projection, scatter
back (non-anchor rows of the output are zero; keys are anchors only).

Sharding: core c handles batch c//2 and HEAD GROUP c%2 (4 of 8 heads).
Both cores of a pair see the same gathered anchor tokens; each computes
q/k/v and attention for its 4 heads over ALL anchors, then a partial
out-projection (sum over its heads). The host adds the two partials
(+ output bias) — out-projection is linear in heads, so no collective
is needed.

Device layout (contraction dims on partitions):
  zT   per 512-token chunk: (128, 6, 512)  z = (x-mu)*rstd, d on partitions
  qT   (128, 4, NA)  per head 128 rows: 96 hd + row96 == 1.0 (mask helper)
  kT   (128, 4, NA)  per head 128 rows: 96 hd + row96 == key-pad mask
  v    (128, T, 4, 128) plain layout: 96 head dims + ones col + zero pad
  scores^T (tk, tq) per (head, 2-tile key group, query-chunk); one
  1024-wide exp per group; probs = exp(scale*s)
  avT  (128, NQH) accumulated over tk; row 96 = softmax denominator
  outT (768, NA) = sum_h Wo_h^T @ (avT_h / denom_h)   [bias added on host]

LayerNorm's affine (ln_g, ln_b) is folded into the weights on the host:
W~ = W * g, bias~ = W @ b + bias.
"""

import numpy as np
import ml_dtypes

import concourse.bass as bass
import concourse.mybir as mybir
import concourse.tile as tile
from concourse import bacc
from concourse.bass_utils import run_bass_kernel_spmd

BF16 = ml_dtypes.bfloat16
F32 = mybir.dt.float32
BF = mybir.dt.bfloat16

B, S, D, H, HD = 4, 2048, 768, 8, 96
HL = H // 2           # heads per core
J = D // 128          # contraction blocks
EPS = 1e-5
SCALE = 1.0 / np.sqrt(HD)
MASK_NEG = -800.0     # exp(SCALE*(qk+MASK_NEG)) ~ 4e-36 (and keeps the
                      # Schraudolph int16 path positive / unsaturated)
LOG2E = 1.4426950408889634
SCH_A = SCALE * 128.0 * LOG2E        # bf16-bits = round(score*SCH_A + SCH_B)
SCH_B = (127.0 - 0.06) * 128.0       # c=0.06 minimizes rms rel err (~1.8%)


def _chunks(total, step):
    out = []
    c = 0
    while c < total:
        out.append((c, min(step, total - c)))
        c += step
    return out


def build(NA, QC):
    """Build the per-core Bacc graph for padded anchor count NA."""
    assert NA % 128 == 0 and QC % 64 == 0 and QC <= NA
    T = NA // 128
    # attention query units (<= 512 wide); the ragged tail chunk (if any)
    # is processed FIRST so its poorly-pipelined dribble overlaps the next
    # chunk's attention instead of serializing at the end of the kernel.
    QSPLIT = _chunks(QC, 512)
    if len(QSPLIT) > 1 and QSPLIT[-1][1] < 512:
        QSPLIT = [QSPLIT[-1]] + QSPLIT[:-1]
    # token chunks: a short first chunk lets the K/Q projections start
    # before LayerNorm (the prologue critical path) finishes later tiles.
    if NA > 256:
        CH = [(0, 256)] + [(c0 + 256, cw) for (c0, cw) in _chunks(NA - 256, 512)]
    else:
        CH = [(0, NA)]

    nc = bacc.Bacc("TRN2", target_bir_lowering=False, debug=False, num_devices=8)

    x_ext = nc.dram_tensor("x", [NA, D], BF, kind="ExternalInput").ap()
    wq_ext = nc.dram_tensor("wq", [128, J * HL * 128], BF, kind="ExternalInput").ap()
    wk_ext = nc.dram_tensor("wk", [128, J * HL * 128], BF, kind="ExternalInput").ap()
    wv_ext = nc.dram_tensor("wv", [128, J * HL * 96], BF, kind="ExternalInput").ap()
    wo_ext = nc.dram_tensor("wo", [128, HL * D], BF, kind="ExternalInput").ap()
    bq_ext = nc.dram_tensor("bq", [128, HL], F32, kind="ExternalInput").ap()
    bk_ext = nc.dram_tensor("bk", [128, HL], F32, kind="ExternalInput").ap()
    bv_ext = nc.dram_tensor("bv", [HL * 96], F32, kind="ExternalInput").ap()
    km_ext = nc.dram_tensor("km", [1, NA], BF, kind="ExternalInput").ap()
    out_ext = nc.dram_tensor("out", [D, QC], BF, kind="ExternalOutput").ap()

    with tile.TileContext(nc) as tc:
        with (
            tc.tile_pool(name="singles", bufs=1) as singles,
            tc.tile_pool(name="work", bufs=5) as work,
            tc.tile_pool(name="probs", bufs=6) as probs_pool,
        ):
            # ---- x first (LN needs it immediately; queues are FIFO so
            # anything emitted before it would delay the whole prologue).
            # Two halves so LN can start after the first 4 tiles land.
            x_all = singles.tile([128, T, D], BF)
            x_v = x_ext.rearrange("(t p) d -> p t d", p=128)
            for (i0, iw) in _chunks(T, 2):
                nc.sync.dma_start(
                    out=x_all[:, i0:i0 + iw, :], in_=x_v[:, i0:i0 + iw, :])
            x_tiles = [x_all[:, i, :] for i in range(T)]

            # ---- weights / constants into SBUF (one DMA per tensor, issued
            # in consumption order: K first, Wo last).
            wq_sb = singles.tile([128, J, HL * 128], BF)
            wk_sb = singles.tile([128, J, HL * 128], BF)
            wv_sb = singles.tile([128, J, HL * 96], BF)
            wo_sb = singles.tile([128, HL, D], BF)
            nc.sync.dma_start(
                out=wk_sb, in_=wk_ext.rearrange("p (j e) -> p j e", j=J))
            nc.sync.dma_start(
                out=wq_sb, in_=wq_ext.rearrange("p (j e) -> p j e", j=J))
            nc.sync.dma_start(
                out=wv_sb, in_=wv_ext.rearrange("p (j e) -> p j e", j=J))
            nc.sync.dma_start(out=wo_sb, in_=wo_ext)
            bq_sb = singles.tile([128, HL], F32)
            nc.gpsimd.dma_start(out=bq_sb, in_=bq_ext)
            bk_sb = singles.tile([128, HL], F32)
            nc.gpsimd.dma_start(out=bk_sb, in_=bk_ext)
            bv_sb = singles.tile([128, HL * 96], F32)
            bv_bcast = bass.AP(
                tensor=bv_ext.tensor, offset=bv_ext.offset,
                ap=[[0, 128], [1, HL * 96]],
            )
            nc.gpsimd.dma_start(out=bv_sb, in_=bv_bcast)

            ones96 = singles.tile([1, 96], BF)
            nc.vector.memset(ones96, 1.0)
            eps_sb = singles.tile([128, 1], F32)
            nc.vector.memset(eps_sb, EPS)
            ident = singles.tile([128, 128], BF)
            from concourse.masks import make_identity
            make_identity(nc, ident)
            zT = [singles.tile([128, J, cw], BF, name=f"zT{c}")
                  for c, (c0, cw) in enumerate(CH)]

            def zt_slice(j, c0, cw):
                ci = max(i for i, (cc0, _) in enumerate(CH) if cc0 <= c0)
                off = c0 - CH[ci][0]
                assert off + cw <= CH[ci][1]
                return zT[ci][:, j, off:off + cw]

            kT = singles.tile([128, HL, NA], BF)
            qT = singles.tile([128, HL, QC], BF)
            v_sb = singles.tile([128, T, HL, 128], BF)
            avn = singles.tile([128, HL, QC], BF)
            nc.gpsimd.memset(avn[96:128, :, :], 0.0)

            # v columns: 0..95 head dims, 96 ones (denominator), 97.. zero
            # (padding to 128 weights keeps FWL on for the av matmuls)
            nc.vector.memset(v_sb[:, :, :, 96:97], 1.0)
            nc.gpsimd.memset(v_sb[:, :, :, 97:128], 0.0)

            with (
                tc.tile_pool(name="ps_proj", bufs=2, space="PSUM") as ps_proj,
                tc.tile_pool(name="ps_t", bufs=3, space="PSUM") as ps_t,
            ):
                # Pipeline per 512-token chunk: LN/z -> transpose (on the
                # otherwise-idle PE) -> K/Q/V projections for that chunk.
                for ci, (c0, cw) in enumerate(CH):
                    tlo, thi = c0 // 128, (c0 + cw) // 128
                    for i in range(tlo, thi):
                        x_i = x_tiles[i]
                        x_g = x_i.rearrange("p (n f) -> p n f", f=384)
                        stats = work.tile([128, 2, 6], F32, tag="stats")
                        for g in range(2):
                            nc.vector.bn_stats(out=stats[:, g, :], in_=x_g[:, g, :])
                        mv = work.tile([128, 2], F32, tag="mv")
                        nc.vector.bn_aggr(out=mv, in_=stats)
                        sd = work.tile([128, 1], F32, tag="sd")
                        nc.scalar.activation(
                            out=sd, in_=mv[:, 1:2],
                            func=mybir.ActivationFunctionType.Sqrt,
                            bias=eps_sb, scale=1.0,
                        )
                        rstd = work.tile([128, 1], F32, tag="rstd")
                        nc.vector.reciprocal(out=rstd, in_=sd)
                        z_i = work.tile([128, D], BF, tag="z")
                        nc.vector.tensor_scalar(
                            out=z_i, in0=x_i,
                            scalar1=mv[:, 0:1], scalar2=rstd,
                            op0=mybir.AluOpType.subtract, op1=mybir.AluOpType.mult,
                        )
                        ioff = (i - tlo) * 128
                        tp = ps_t.tile([128, J, 128], BF, tag="tp")
                        for j in range(J):
                            nc.tensor.transpose(
                                tp[:, j, :], z_i[:, j * 128:(j + 1) * 128],
                                ident)
                        nc.scalar.activation(
                            out=zT[ci][:, :, ioff:ioff + 128], in_=tp,
                            func=mybir.ActivationFunctionType.Copy,
                        )

                    # K / Q projections for this chunk (local heads).
                    # Q only covers [0, QC) — queries past the last anchor
                    # are never read.
                    qw_c = min(cw, max(0, QC - c0))
                    for (name, w_sb, b_sb, dst, ncols) in (
                        ("k", wk_sb, bk_sb, kT, cw),
                        ("q", wq_sb, bq_sb, qT, qw_c),
                    ):
                        if ncols == 0:
                            continue
                        for m in range(HL):
                            ps = ps_proj.tile([128, ncols], F32, tag="proj")
                            for j in range(J):
                                nc.tensor.matmul(
                                    ps,
                                    lhsT=w_sb[:, j, m * 128:(m + 1) * 128],
                                    rhs=zT[ci][:, j, :ncols],
                                    start=(j == 0), stop=(j == J - 1),
                                )
                            nc.vector.tensor_scalar_add(
                                out=dst[:, m, c0:c0 + ncols], in0=ps,
                                scalar1=b_sb[:, m:m + 1],
                            )
                    # V projection for this chunk's token tiles (all 4 local
                    # heads in one N=384 matmul per contraction block)
                    for i in range(tlo, thi):
                        ps = ps_proj.tile([128, HL * 96], F32, tag="proj")
                        for j in range(J):
                            nc.tensor.matmul(
                                ps,
                                lhsT=zt_slice(j, i * 128, 128),
                                rhs=wv_sb[:, j, :],
                                start=(j == 0), stop=(j == J - 1),
                            )
                        nc.vector.tensor_tensor(
                            out=v_sb[:, i, :, 0:96],
                            in0=ps.rearrange("p (h c) -> p h c", c=96),
                            in1=bv_sb.rearrange("p (h c) -> p h c", c=96),
                            op=mybir.AluOpType.add,
                        )

                # overwrite kT row 96 of every head with the key-pad mask row
                km_bcast = bass.AP(
                    tensor=km_ext.tensor, offset=km_ext.offset,
                    ap=[[0, 1], [0, HL], [1, NA]],
                )
                nc.gpsimd.dma_start(out=kT[96:97, :, :], in_=km_bcast)

            # ---- attention + out-projection, fused per query chunk.
            # Per (query-chunk, head): scores for 2 key-tiles land in a
            # 2-bank PSUM group; ONE 1024-wide exp per group (halves the
            # per-instruction overhead on ScalarE — the phase bottleneck).
            # av matmuls for group g are emitted right after exp g, so the
            # in-order TensorE executes them while exp g+1 runs. After all
            # heads of a chunk, that chunk's out-projection is emitted; its
            # matmuls + output DMA overlap the next chunk's attention.
            # PSUM budget: scores 2x2 + av 2 + out-proj 2 = 8 banks.
            TKG = _chunks(T, 2)

            with (
                tc.tile_pool(name="ps_s", bufs=2, space="PSUM") as ps_s,
                tc.tile_pool(name="ps_av", bufs=2, space="PSUM") as ps_av,
                tc.tile_pool(name="ps_o", bufs=2, space="PSUM") as ps_o,
            ):
                def emit_av(h, grp, av_ps, qw):
                    t0, tn, pb = grp
                    for gi in range(tn):
                        tk = t0 + gi
                        nc.tensor.matmul(
                            av_ps[:, :qw],
                            lhsT=v_sb[:, tk, h, :],
                            rhs=pb[:, gi, :qw],
                            start=(tk == 0), stop=(tk == T - 1),
                            skip_group_check=True,
                        )

                def tail(h, q0, qw, av_ps):
                    # normalize: avn = avT[0:96] * (1 / avT[96]) broadcast.
                    # (recip_approx_fast is a bitwise-seed custom op — needs
                    # its input in SBUF, so copy the denominator row first.)
                    d_sb = work.tile([1, qw], F32, tag="dsb")
                    nc.vector.tensor_copy(out=d_sb, in_=av_ps[96:97, :qw])
                    rec32 = work.tile([1, qw], F32, tag="rec32")
                    nc.vector.reciprocal_approx_fast(out=rec32, in_=d_sb)
                    recip_bf = work.tile([1, qw], BF, tag="recipbf")
                    nc.vector.tensor_copy(out=recip_bf, in_=rec32)
                    bc_sb = work.tile([96, qw], BF, tag="bc")
                    nc.gpsimd.partition_broadcast(out_ap=bc_sb, in_ap=recip_bf)
                    nc.vector.tensor_tensor(
                        out=avn[0:96, h, q0:q0 + qw],
                        in0=av_ps[0:96, :qw], in1=bc_sb,
                        op=mybir.AluOpType.mult,
                    )

                # exp groups handled by DVE (Schraudolph bf16 bit-trick,
                # ~1.8% rms) instead of ScalarE's exact LUT exp: every 3rd
                # group starting at 1 (so ~1/3 of the exp stream moves off
                # the ScalarE bottleneck; error contribution ~1%).
                ngrp = len(TKG)
                dve_groups = {g for g in range(1, ngrp, 3)} if ngrp >= 3 else set()

                for (q0, qw) in QSPLIT:
                    for h in range(HL):
                        av_ps = ps_av.tile([128, 512], F32, tag="av")
                        pend = None
                        for g, (t0, tn) in enumerate(TKG):
                            s_ps = ps_s.tile([128, 2, 512], F32, tag="s")
                            for gi in range(tn):
                                tk = t0 + gi
                                nc.tensor.matmul(
                                    s_ps[:, gi, :qw],
                                    lhsT=kT[:, h, tk * 128:(tk + 1) * 128],
                                    rhs=qT[:, h, q0:q0 + qw],
                                    start=True, stop=True,
                                )
                            pb = probs_pool.tile([128, 2, 512], BF, tag="p")
                            if g in dve_groups:
                                nc.vector.tensor_scalar(
                                    out=pb.bitcast(mybir.dt.int16)[:, :tn, :qw],
                                    in0=s_ps[:, :tn, :qw],
                                    scalar1=float(SCH_A), scalar2=float(SCH_B),
                                    op0=mybir.AluOpType.mult,
                                    op1=mybir.AluOpType.add,
                                )
                            else:
                                nc.scalar.activation(
                                    out=pb[:, :tn, :qw], in_=s_ps[:, :tn, :qw],
                                    func=mybir.ActivationFunctionType.Exp,
                                    scale=float(SCALE),
                                )
                            if pend is not None:
                                emit_av(h, pend, av_ps, qw)
                            pend = (t0, tn, pb)
                        emit_av(h, pend, av_ps, qw)
                        tail(h, q0, qw, av_ps)

                    # partial out projection for this query chunk (sum over
                    # local heads; host adds the pair partials + bias)
                    for m in range(J):
                        o_ps = ps_o.tile([128, 512], F32, tag="o")
                        for hh in range(HL):
                            nc.tensor.matmul(
                                o_ps[:, :qw],
                                lhsT=wo_sb[:, hh, m * 128:(m + 1) * 128],
                                rhs=avn[:, hh, q0:q0 + qw],
                                start=(hh == 0), stop=(hh == HL - 1),
                            )
                        o_sb = work.tile([128, 512], BF, tag="osb")
                        # ScalarE evacuates (DVE is loaded with Schraudolph
                        # exps + tails in this phase)
                        nc.scalar.activation(
                            out=o_sb[:, :qw], in_=o_ps[:, :qw],
                            func=mybir.ActivationFunctionType.Copy,
                        )
                        eng = nc.sync if (m % 2 == 0) else nc.gpsimd
                        eng.dma_start(
                            out=out_ext[m * 128:(m + 1) * 128, q0:q0 + qw],
                            in_=o_sb[:, :qw],
                        )

    nc.compile()
    return nc


_CACHE = {}


def _prep_weights(ln_g, ln_b, Wq, bq, Wk, bk, Wv, bv, Wo, bo):
    """Per-head-group device weight layouts. Returns [group0, group1]."""
    def pad_head_T(W, hg):
        # (W * g).T for heads of the group, padded 96 -> 128 cols, then
        # SBUF layout (128, J, HL*128): [p, j, e] = WT[j*128+p, e]
        WT = (W * ln_g[None, :]).T.astype(np.float32)
        WT = WT.reshape(D, H, 96)[:, hg * HL:(hg + 1) * HL, :]
        Wp = np.zeros((D, HL, 128), np.float32)
        Wp[:, :, :96] = WT
        Wp = Wp.reshape(J, 128, HL * 128).transpose(1, 0, 2)
        return np.ascontiguousarray(Wp.reshape(128, J * HL * 128)).astype(BF16)

    def plain_T(W, hg):
        WT = (W * ln_g[None, :]).T.astype(np.float32)
        WT = WT.reshape(D, H, 96)[:, hg * HL:(hg + 1) * HL, :].reshape(D, HL * 96)
        Wp = WT.reshape(J, 128, HL * 96).transpose(1, 0, 2)
        return np.ascontiguousarray(Wp.reshape(128, J * HL * 96)).astype(BF16)

    def pad_bias(bb, hg, ones_row):
        bp = np.zeros((HL, 128), np.float32)
        bp[:, :96] = bb.reshape(H, 96)[hg * HL:(hg + 1) * HL]
        if ones_row:
            bp[:, 96] = 1.0
        return np.ascontiguousarray(bp.T).astype(np.float32)  # (128, HL)

    def pad_wo(hg):
        w = np.zeros((128, HL, D), np.float32)
        w[:96] = Wo.T.reshape(H, 96, D)[hg * HL:(hg + 1) * HL].transpose(1, 0, 2)
        return np.ascontiguousarray(w.reshape(128, HL * D)).astype(BF16)

    bbq = Wq @ ln_b + bq
    bbk = Wk @ ln_b + bk
    bbv = Wv @ ln_b + bv
    return [{
        "wq": pad_head_T(Wq, hg),
        "wk": pad_head_T(Wk, hg),
        "wv": plain_T(Wv, hg),
        "wo": pad_wo(hg),
        "bq": pad_bias(bbq, hg, True),
        "bk": pad_bias(bbk, hg, False),
        "bv": np.ascontiguousarray(
            bbv.reshape(H, 96)[hg * HL:(hg + 1) * HL].reshape(-1)
        ).astype(np.float32),
    } for hg in range(2)]


def _make_in_maps(hidden_states, idx, NA, wmaps):
    in_maps = []
    for c in range(8):
        b, hg = c // 2, c % 2
        nb = len(idx[b])
        xg = np.zeros((NA, D), np.float32)
        xg[:nb] = hidden_states[b][idx[b]]
        km = np.zeros((NA,), np.float32)
        km[nb:] = MASK_NEG
        in_maps.append({
            "x": xg.astype(BF16),
            "km": km.reshape(1, NA).astype(BF16),
            **wmaps[hg],
        })
    return in_maps


def kernel(hidden_states, anchor_mask, ln_g, ln_b,
           Wq, bq, Wk, bk, Wv, bv, Wo, bo):
    hidden_states = np.asarray(hidden_states, dtype=np.float32)
    anchor_mask = np.asarray(anchor_mask).astype(bool)
    args = [np.asarray(a, dtype=np.float32)
            for a in (ln_g, ln_b, Wq, bq, Wk, bk, Wv, bv, Wo, bo)]
    bo_f = args[-1]

    idx = [np.nonzero(anchor_mask[b])[0] for b in range(B)]
    max_nb = max(len(i) for i in idx)
    NA = max(256, ((max_nb + 127) // 128) * 128)
    QC = max(128, ((max_nb + 63) // 64) * 64)

    if (NA, QC) not in _CACHE:
        _CACHE[(NA, QC)] = build(NA, QC)
    nc = _CACHE[(NA, QC)]

    wmaps = _prep_weights(*args)
    in_maps = _make_in_maps(hidden_states, idx, NA, wmaps)

    res = run_bass_kernel_spmd(nc, in_maps, core_ids=list(range(8)))

    out = np.zeros((B, S, D), np.float32)
    for b in range(B):
        nb = len(idx[b])
        oT = (res.results[2 * b]["out"].astype(np.float32)
              + res.results[2 * b + 1]["out"].astype(np.float32))
        out[b, idx[b]] = oT.T[:nb] + bo_f[None, :]
    return out



# revision 18
# speedup vs baseline: 1.0126x; 1.0126x over previous
"""AnchorAttention Trainium2 kernel (8 NeuronCores, SPMD, no collectives).

Math (per batch): gather anchor rows of hidden_states, LayerNorm, QKV
projections, dense attention among anchors only, out-projection, scatter
back (non-anchor rows of the output are zero; keys are anchors only).

Sharding: core c handles batch c//2 and HEAD GROUP c%2 (4 of 8 heads).
Both cores of a pair see the same gathered anchor tokens; each computes
q/k/v and attention for its 4 heads over ALL anchors, then a partial
out-projection (sum over its heads). The host adds the two partials
(+ output bias) — out-projection is linear in heads, so no collective
is needed.

Device layout (contraction dims on partitions):
  zT   per 512-token chunk: (128, 6, 512)  z = (x-mu)*rstd, d on partitions
  qT   (128, 4, NA)  per head 128 rows: 96 hd + row96 == 1.0 (mask helper)
  kT   (128, 4, NA)  per head 128 rows: 96 hd + row96 == key-pad mask
  v    (128, T, 4, 128) plain layout: 96 head dims + ones col + zero pad
  scores^T (tk, tq) per (head, 2-tile key group, query-chunk); one
  1024-wide exp per group; probs = exp(scale*s)
  avT  (128, NQH) accumulated over tk; row 96 = softmax denominator
  outT (768, NA) = sum_h Wo_h^T @ (avT_h / denom_h)   [bias added on host]

LayerNorm's affine (ln_g, ln_b) is folded into the weights on the host:
W~ = W * g, bias~ = W @ b + bias.
"""

import numpy as np
import ml_dtypes

import concourse.bass as bass
import concourse.mybir as mybir
import concourse.tile as tile
from concourse import bacc
from concourse.bass_utils import run_bass_kernel_spmd

BF16 = ml_dtypes.bfloat16
F32 = mybir.dt.float32
BF = mybir.dt.bfloat16

B, S, D, H, HD = 4, 2048, 768, 8, 96
HL = H // 2           # heads per core
J = D // 128          # contraction blocks
EPS = 1e-5
SCALE = 1.0 / np.sqrt(HD)
MASK_NEG = -800.0     # exp(SCALE*(qk+MASK_NEG)) ~ 4e-36 (and keeps the
                      # Schraudolph int16 path positive / unsaturated)
LOG2E = 1.4426950408889634
SCH_A = SCALE * 128.0 * LOG2E        # bf16-bits = round(score*SCH_A + SCH_B)
SCH_B = (127.0 - 0.06) * 128.0       # c=0.06 minimizes rms rel err (~1.8%)


def _chunks(total, step):
    out = []
    c = 0
    while c < total:
        out.append((c, min(step, total - c)))
        c += step
    return out


def build(NA, QC):
    """Build the per-core Bacc graph for padded anchor count NA."""
    assert NA % 128 == 0 and QC % 64 == 0 and QC <= NA
    T = NA // 128
    # attention query units (<= 512 wide); the ragged tail chunk (if any)
    # is processed FIRST so its poorly-pipelined dribble overlaps the next
    # chunk's attention instead of serializing at the end of the kernel.
    QSPLIT = _chunks(QC, 512)
    if len(QSPLIT) > 1 and QSPLIT[-1][1] < 512:
        QSPLIT = [QSPLIT[-1]] + QSPLIT[:-1]
    # token chunks: a short first chunk lets the K/Q projections start
    # before LayerNorm (the prologue critical path) finishes later tiles.
    if NA > 256:
        CH = [(0, 256)] + [(c0 + 256, cw) for (c0, cw) in _chunks(NA - 256, 512)]
    else:
        CH = [(0, NA)]

    nc = bacc.Bacc("TRN2", target_bir_lowering=False, debug=False, num_devices=8)

    x_ext = nc.dram_tensor("x", [NA, D], BF, kind="ExternalInput").ap()
    wq_ext = nc.dram_tensor("wq", [128, J * HL * 128], BF, kind="ExternalInput").ap()
    wk_ext = nc.dram_tensor("wk", [128, J * HL * 128], BF, kind="ExternalInput").ap()
    wv_ext = nc.dram_tensor("wv", [128, J * HL * 96], BF, kind="ExternalInput").ap()
    wo_ext = nc.dram_tensor("wo", [128, HL * D], BF, kind="ExternalInput").ap()
    bq_ext = nc.dram_tensor("bq", [128, HL], F32, kind="ExternalInput").ap()
    bk_ext = nc.dram_tensor("bk", [128, HL], F32, kind="ExternalInput").ap()
    bv_ext = nc.dram_tensor("bv", [HL * 96], F32, kind="ExternalInput").ap()
    km_ext = nc.dram_tensor("km", [1, NA], BF, kind="ExternalInput").ap()
    out_ext = nc.dram_tensor("out", [D, QC], BF, kind="ExternalOutput").ap()

    with tile.TileContext(nc) as tc:
        with (
            tc.tile_pool(name="singles", bufs=1) as singles,
            tc.tile_pool(name="work", bufs=5) as work,
            tc.tile_pool(name="probs", bufs=6) as probs_pool,
        ):
            # ---- x first (LN needs it immediately; queues are FIFO so
            # anything emitted before it would delay the whole prologue).
            # Two halves so LN can start after the first 4 tiles land.
            x_all = singles.tile([128, T, D], BF)
            x_v = x_ext.rearrange("(t p) d -> p t d", p=128)
            for (i0, iw) in _chunks(T, 2):
                nc.sync.dma_start(
                    out=x_all[:, i0:i0 + iw, :], in_=x_v[:, i0:i0 + iw, :])
            x_tiles = [x_all[:, i, :] for i in range(T)]

            # ---- weights / constants into SBUF (one DMA per tensor, issued
            # in consumption order: K first, Wo last).
            wq_sb = singles.tile([128, J, HL * 128], BF)
            wk_sb = singles.tile([128, J, HL * 128], BF)
            wv_sb = singles.tile([128, J, HL * 96], BF)
            wo_sb = singles.tile([128, HL, D], BF)
            nc.sync.dma_start(
                out=wk_sb, in_=wk_ext.rearrange("p (j e) -> p j e", j=J))
            nc.sync.dma_start(
                out=wq_sb, in_=wq_ext.rearrange("p (j e) -> p j e", j=J))
            nc.sync.dma_start(
                out=wv_sb, in_=wv_ext.rearrange("p (j e) -> p j e", j=J))
            nc.sync.dma_start(out=wo_sb, in_=wo_ext)
            bq_sb = singles.tile([128, HL], F32)
            nc.gpsimd.dma_start(out=bq_sb, in_=bq_ext)
            bk_sb = singles.tile([128, HL], F32)
            nc.gpsimd.dma_start(out=bk_sb, in_=bk_ext)
            bv_sb = singles.tile([128, HL * 96], F32)
            bv_bcast = bass.AP(
                tensor=bv_ext.tensor, offset=bv_ext.offset,
                ap=[[0, 128], [1, HL * 96]],
            )
            nc.gpsimd.dma_start(out=bv_sb, in_=bv_bcast)

            ones96 = singles.tile([1, 96], BF)
            nc.vector.memset(ones96, 1.0)
            eps_sb = singles.tile([128, 1], F32)
            nc.vector.memset(eps_sb, EPS)
            ident = singles.tile([128, 128], BF)
            from concourse.masks import make_identity
            make_identity(nc, ident)
            zT = [singles.tile([128, J, cw], BF, name=f"zT{c}")
                  for c, (c0, cw) in enumerate(CH)]

            def zt_slice(j, c0, cw):
                ci = max(i for i, (cc0, _) in enumerate(CH) if cc0 <= c0)
                off = c0 - CH[ci][0]
                assert off + cw <= CH[ci][1]
                return zT[ci][:, j, off:off + cw]

            kT = singles.tile([128, HL, NA], BF)
            qT = singles.tile([128, HL, QC], BF)
            v_sb = singles.tile([128, T, HL, 128], BF)
            avn = singles.tile([128, HL, QC], BF)
            nc.gpsimd.memset(avn[96:128, :, :], 0.0)

            # v columns: 0..95 head dims, 96 ones (denominator), 97.. zero
            # (padding to 128 weights keeps FWL on for the av matmuls)
            nc.vector.memset(v_sb[:, :, :, 96:97], 1.0)
            nc.gpsimd.memset(v_sb[:, :, :, 97:128], 0.0)

            with (
                tc.tile_pool(name="ps_proj", bufs=2, space="PSUM") as ps_proj,
                tc.tile_pool(name="ps_t", bufs=3, space="PSUM") as ps_t,
            ):
                # Pipeline per 512-token chunk: LN/z -> transpose (on the
                # otherwise-idle PE) -> K/Q/V projections for that chunk.
                for ci, (c0, cw) in enumerate(CH):
                    tlo, thi = c0 // 128, (c0 + cw) // 128
                    for i in range(tlo, thi):
                        x_i = x_tiles[i]
                        x_g = x_i.rearrange("p (n f) -> p n f", f=384)
                        stats = work.tile([128, 2, 6], F32, tag="stats")
                        for g in range(2):
                            nc.vector.bn_stats(out=stats[:, g, :], in_=x_g[:, g, :])
                        mv = work.tile([128, 2], F32, tag="mv")
                        nc.vector.bn_aggr(out=mv, in_=stats)
                        sd = work.tile([128, 1], F32, tag="sd")
                        nc.scalar.activation(
                            out=sd, in_=mv[:, 1:2],
                            func=mybir.ActivationFunctionType.Sqrt,
                            bias=eps_sb, scale=1.0,
                        )
                        rstd = work.tile([128, 1], F32, tag="rstd")
                        nc.vector.reciprocal(out=rstd, in_=sd)
                        z_i = work.tile([128, D], BF, tag="z")
                        nc.vector.tensor_scalar(
                            out=z_i, in0=x_i,
                            scalar1=mv[:, 0:1], scalar2=rstd,
                            op0=mybir.AluOpType.subtract, op1=mybir.AluOpType.mult,
                        )
                        ioff = (i - tlo) * 128
                        tp = ps_t.tile([128, J, 128], BF, tag="tp")
                        for j in range(J):
                            nc.tensor.transpose(
                                tp[:, j, :], z_i[:, j * 128:(j + 1) * 128],
                                ident)
                        nc.scalar.activation(
                            out=zT[ci][:, :, ioff:ioff + 128], in_=tp,
                            func=mybir.ActivationFunctionType.Copy,
                        )

                    # K / Q projections for this chunk (local heads).
                    # Q only covers [0, QC) — queries past the last anchor
                    # are never read.
                    qw_c = min(cw, max(0, QC - c0))
                    for (name, w_sb, b_sb, dst, ncols) in (
                        ("k", wk_sb, bk_sb, kT, cw),
                        ("q", wq_sb, bq_sb, qT, qw_c),
                    ):
                        if ncols == 0:
                            continue
                        for m in range(HL):
                            ps = ps_proj.tile([128, ncols], F32, tag="proj")
                            for j in range(J):
                                nc.tensor.matmul(
                                    ps,
                                    lhsT=w_sb[:, j, m * 128:(m + 1) * 128],
                                    rhs=zT[ci][:, j, :ncols],
                                    start=(j == 0), stop=(j == J - 1),
                                )
                            nc.vector.tensor_scalar_add(
                                out=dst[:, m, c0:c0 + ncols], in0=ps,
                                scalar1=b_sb[:, m:m + 1],
                            )
                    # V projection for this chunk's token tiles (all 4 local
                    # heads in one N=384 matmul per contraction block)
                    for i in range(tlo, thi):
                        ps = ps_proj.tile([128, HL * 96], F32, tag="proj")
                        for j in range(J):
                            nc.tensor.matmul(
                                ps,
                                lhsT=zt_slice(j, i * 128, 128),
                                rhs=wv_sb[:, j, :],
                                start=(j == 0), stop=(j == J - 1),
                            )
                        nc.vector.tensor_tensor(
                            out=v_sb[:, i, :, 0:96],
                            in0=ps.rearrange("p (h c) -> p h c", c=96),
                            in1=bv_sb.rearrange("p (h c) -> p h c", c=96),
                            op=mybir.AluOpType.add,
                        )

                # overwrite kT row 96 of every head with the key-pad mask row
                km_bcast = bass.AP(
                    tensor=km_ext.tensor, offset=km_ext.offset,
                    ap=[[0, 1], [0, HL], [1, NA]],
                )
                nc.gpsimd.dma_start(out=kT[96:97, :, :], in_=km_bcast)

            # ---- attention + out-projection, fused per query chunk.
            # Per (query-chunk, head): scores for 2 key-tiles land in a
            # 2-bank PSUM group; ONE 1024-wide exp per group (halves the
            # per-instruction overhead on ScalarE — the phase bottleneck).
            # av matmuls for group g are emitted right after exp g, so the
            # in-order TensorE executes them while exp g+1 runs. After all
            # heads of a chunk, that chunk's out-projection is emitted; its
            # matmuls + output DMA overlap the next chunk's attention.
            # PSUM budget: scores 2x2 + av 2 + out-proj 2 = 8 banks.
            TKG = _chunks(T, 2)

            with (
                tc.tile_pool(name="ps_s", bufs=2, space="PSUM") as ps_s,
                tc.tile_pool(name="ps_av", bufs=2, space="PSUM") as ps_av,
                tc.tile_pool(name="ps_o", bufs=2, space="PSUM") as ps_o,
            ):
                def emit_av(h, grp, av_ps, qw):
                    t0, tn, pb = grp
                    for gi in range(tn):
                        tk = t0 + gi
                        nc.tensor.matmul(
                            av_ps[:, :qw],
                            lhsT=v_sb[:, tk, h, :],
                            rhs=pb[:, gi, :qw],
                            start=(tk == 0), stop=(tk == T - 1),
                            skip_group_check=True,
                        )

                def tail(h, q0, qw, av_ps):
                    # normalize: avn = avT[0:96] * (1 / avT[96]) broadcast.
                    # (recip_approx_fast is a bitwise-seed custom op — needs
                    # its input in SBUF, so copy the denominator row first.)
                    d_sb = work.tile([1, qw], F32, tag="dsb")
                    nc.vector.tensor_copy(out=d_sb, in_=av_ps[96:97, :qw])
                    rec32 = work.tile([1, qw], F32, tag="rec32")
                    nc.vector.reciprocal_approx_fast(out=rec32, in_=d_sb)
                    recip_bf = work.tile([1, qw], BF, tag="recipbf")
                    nc.vector.tensor_copy(out=recip_bf, in_=rec32)
                    bc_sb = work.tile([96, qw], BF, tag="bc")
                    nc.gpsimd.partition_broadcast(out_ap=bc_sb, in_ap=recip_bf)
                    nc.vector.tensor_tensor(
                        out=avn[0:96, h, q0:q0 + qw],
                        in0=av_ps[0:96, :qw], in1=bc_sb,
                        op=mybir.AluOpType.mult,
                    )

                # exp groups handled by DVE (Schraudolph bf16 bit-trick,
                # ~1.8% rms) instead of ScalarE's exact LUT exp: every 3rd
                # group starting at 1 (so ~1/3 of the exp stream moves off
                # the ScalarE bottleneck; error contribution ~1%).
                ngrp = len(TKG)
                dve_groups = {g for g in range(1, ngrp, 3)} if ngrp >= 3 else set()

                def emit_outproj(q0, qw):
                    # partial out projection for one query chunk (sum over
                    # local heads; host adds the pair partials + bias)
                    for m in range(J):
                        o_ps = ps_o.tile([128, 512], F32, tag="o")
                        for hh in range(HL):
                            nc.tensor.matmul(
                                o_ps[:, :qw],
                                lhsT=wo_sb[:, hh, m * 128:(m + 1) * 128],
                                rhs=avn[:, hh, q0:q0 + qw],
                                start=(hh == 0), stop=(hh == HL - 1),
                            )
                        o_sb = work.tile([128, 512], BF, tag="osb")
                        # ScalarE evacuates (DVE is loaded with Schraudolph
                        # exps + tails in this phase)
                        nc.scalar.activation(
                            out=o_sb[:, :qw], in_=o_ps[:, :qw],
                            func=mybir.ActivationFunctionType.Copy,
                        )
                        eng = nc.sync if (m % 2 == 0) else nc.gpsimd
                        eng.dma_start(
                            out=out_ext[m * 128:(m + 1) * 128, q0:q0 + qw],
                            in_=o_sb[:, :qw],
                        )

                # Rolling software pipeline across (query-chunk, head)
                # iterations: iteration i's last av group + tail (and, after
                # a chunk's final head, that chunk's out-projection) are
                # emitted inside iteration i+1, after its first exp — so the
                # in-order TensorE always has the next iteration's scores to
                # chew on while ScalarE/VectorE finish iteration i.
                iters = [(q0, qw, h) for (q0, qw) in QSPLIT for h in range(HL)]
                carry = None          # (h, pend_grp, av_ps, q0, qw, oproj)
                for (q0, qw, h) in iters:
                    av_ps = ps_av.tile([128, 512], F32, tag="av")
                    pend = None
                    for g, (t0, tn) in enumerate(TKG):
                        s_ps = ps_s.tile([128, 2, 512], F32, tag="s")
                        for gi in range(tn):
                            tk = t0 + gi
                            nc.tensor.matmul(
                                s_ps[:, gi, :qw],
                                lhsT=kT[:, h, tk * 128:(tk + 1) * 128],
                                rhs=qT[:, h, q0:q0 + qw],
                                start=True, stop=True,
                            )
                        pb = probs_pool.tile([128, 2, 512], BF, tag="p")
                        if g in dve_groups:
                            nc.vector.tensor_scalar(
                                out=pb.bitcast(mybir.dt.int16)[:, :tn, :qw],
                                in0=s_ps[:, :tn, :qw],
                                scalar1=float(SCH_A), scalar2=float(SCH_B),
                                op0=mybir.AluOpType.mult,
                                op1=mybir.AluOpType.add,
                            )
                        else:
                            nc.scalar.activation(
                                out=pb[:, :tn, :qw], in_=s_ps[:, :tn, :qw],
                                func=mybir.ActivationFunctionType.Exp,
                                scale=float(SCALE),
                            )
                        if g == 0 and carry is not None:
                            ch_, cpend, cav, cq0, cqw, coproj = carry
                            emit_av(ch_, cpend, cav, cqw)
                            tail(ch_, cq0, cqw, cav)
                            if coproj:
                                emit_outproj(cq0, cqw)
                            carry = None
                        if pend is not None:
                            emit_av(h, pend, av_ps, qw)
                        pend = (t0, tn, pb)
                    carry = (h, pend, av_ps, q0, qw, h == HL - 1)
                ch_, cpend, cav, cq0, cqw, coproj = carry
                emit_av(ch_, cpend, cav, cqw)
                tail(ch_, cq0, cqw, cav)
                if coproj:
                    emit_outproj(cq0, cqw)

    nc.compile()
    return nc


_CACHE = {}


def _prep_weights(ln_g, ln_b, Wq, bq, Wk, bk, Wv, bv, Wo, bo):
    """Per-head-group device weight layouts. Returns [group0, group1]."""
    def pad_head_T(W, hg):
        # (W * g).T for heads of the group, padded 96 -> 128 cols, then
        # SBUF layout (128, J, HL*128): [p, j, e] = WT[j*128+p, e]
        WT = (W * ln_g[None, :]).T.astype(np.float32)
        WT = WT.reshape(D, H, 96)[:, hg * HL:(hg + 1) * HL, :]
        Wp = np.zeros((D, HL, 128), np.float32)
        Wp[:, :, :96] = WT
        Wp = Wp.reshape(J, 128, HL * 128).transpose(1, 0, 2)
        return np.ascontiguousarray(Wp.reshape(128, J * HL * 128)).astype(BF16)

    def plain_T(W, hg):
        WT = (W * ln_g[None, :]).T.astype(np.float32)
        WT = WT.reshape(D, H, 96)[:, hg * HL:(hg + 1) * HL, :].reshape(D, HL * 96)
        Wp = WT.reshape(J, 128, HL * 96).transpose(1, 0, 2)
        return np.ascontiguousarray(Wp.reshape(128, J * HL * 96)).astype(BF16)

    def pad_bias(bb, hg, ones_row):
        bp = np.zeros((HL, 128), np.float32)
        bp[:, :96] = bb.reshape(H, 96)[hg * HL:(hg + 1) * HL]
        if ones_row:
            bp[:, 96] = 1.0
        return np.ascontiguousarray(bp.T).astype(np.float32)  # (128, HL)

    def pad_wo(hg):
        w = np.zeros((128, HL, D), np.float32)
        w[:96] = Wo.T.reshape(H, 96, D)[hg * HL:(hg + 1) * HL].transpose(1, 0, 2)
        return np.ascontiguousarray(w.reshape(128, HL * D)).astype(BF16)

    bbq = Wq @ ln_b + bq
    bbk = Wk @ ln_b + bk
    bbv = Wv @ ln_b + bv
    return [{
        "wq": pad_head_T(Wq, hg),
        "wk": pad_head_T(Wk, hg),
        "wv": plain_T(Wv, hg),
        "wo": pad_wo(hg),
        "bq": pad_bias(bbq, hg, True),
        "bk": pad_bias(bbk, hg, False),
        "bv": np.ascontiguousarray(
            bbv.reshape(H, 96)[hg * HL:(hg + 1) * HL].reshape(-1)
        ).astype(np.float32),
    } for hg in range(2)]


def _make_in_maps(hidden_states, idx, NA, wmaps):
    in_maps = []
    for c in range(8):
        b, hg = c // 2, c % 2
        nb = len(idx[b])
        xg = np.zeros((NA, D), np.float32)
        xg[:nb] = hidden_states[b][idx[b]]
        km = np.zeros((NA,), np.float32)
        km[nb:] = MASK_NEG
        in_maps.append({
            "x": xg.astype(BF16),
            "km": km.reshape(1, NA).astype(BF16),
            **wmaps[hg],
        })
    return in_maps


def kernel(hidden_states, anchor_mask, ln_g, ln_b,
           Wq, bq, Wk, bk, Wv, bv, Wo, bo):
    hidden_states = np.asarray(hidden_states, dtype=np.float32)
    anchor_mask = np.asarray(anchor_mask).astype(bool)
    args = [np.asarray(a, dtype=np.float32)
            for a in (ln_g, ln_b, Wq, bq, Wk, bk, Wv, bv, Wo, bo)]
    bo_f = args[-1]

    idx = [np.nonzero(anchor_mask[b])[0] for b in range(B)]
    max_nb = max(len(i) for i in idx)
    NA = max(256, ((max_nb + 127) // 128) * 128)
    QC = max(128, ((max_nb + 63) // 64) * 64)

    if (NA, QC) not in _CACHE:
        _CACHE[(NA, QC)] = build(NA, QC)
    nc = _CACHE[(NA, QC)]

    wmaps = _prep_weights(*args)
    in_maps = _make_in_maps(hidden_states, idx, NA, wmaps)

    res = run_bass_kernel_spmd(nc, in_maps, core_ids=list(range(8)))

    out = np.zeros((B, S, D), np.float32)
    for b in range(B):
        nb = len(idx[b])
        oT = (res.results[2 * b]["out"].astype(np.float32)
              + res.results[2 * b + 1]["out"].astype(np.float32))
        out[b, idx[b]] = oT.T[:nb] + bo_f[None, :]
    return out



# revision 25
# speedup vs baseline: 1.0387x; 1.0258x over previous
"""AnchorAttention Trainium2 kernel (8 NeuronCores, SPMD, no collectives).

Math (per batch): gather anchor rows of hidden_states, LayerNorm, QKV
projections, dense attention among anchors only, out-projection, scatter
back (non-anchor rows of the output are zero; keys are anchors only).

Sharding: core c handles batch c//2 and HEAD GROUP c%2 (4 of 8 heads).
Both cores of a pair see the same gathered anchor tokens; each computes
q/k/v and attention for its 4 heads over ALL anchors, then a partial
out-projection (sum over its heads). The host adds the two partials
(+ output bias) — out-projection is linear in heads, so no collective
is needed.

Device layout (contraction dims on partitions):
  zT   per 512-token chunk: (128, 6, 512)  z = (x-mu)*rstd, d on partitions
  qT   (128, 4, NA)  per head 128 rows: 96 hd + row96 == 1.0 (mask helper)
  kT   (128, 4, NA)  per head 128 rows: 96 hd + row96 == key-pad mask
  v    (128, T, 4, 128) plain layout: 96 head dims + ones col + zero pad
  scores^T (tk, tq) per (head, 2-tile key group, query-chunk); one
  1024-wide exp per group; probs = exp(scale*s)
  avT  (128, NQH) accumulated over tk; row 96 = softmax denominator
  outT (768, NA) = sum_h Wo_h^T @ (avT_h / denom_h)   [bias added on host]

LayerNorm's affine (ln_g, ln_b) is folded into the weights on the host:
W~ = W * g, bias~ = W @ b + bias.
"""

import numpy as np
import ml_dtypes

import concourse.bass as bass
import concourse.mybir as mybir
import concourse.tile as tile
from concourse import bacc
from concourse.bass_utils import run_bass_kernel_spmd

BF16 = ml_dtypes.bfloat16
F32 = mybir.dt.float32
BF = mybir.dt.bfloat16

B, S, D, H, HD = 4, 2048, 768, 8, 96
HL = H // 2           # heads per core
J = D // 128          # contraction blocks
EPS = 1e-5
SCALE = 1.0 / np.sqrt(HD)
MASK_NEG = -800.0     # exp(SCALE*(qk+MASK_NEG)) ~ 4e-36 (and keeps the
                      # Schraudolph int16 path positive / unsaturated)
LOG2E = 1.4426950408889634
SCH_A = SCALE * 128.0 * LOG2E        # bf16-bits = round(score*SCH_A + SCH_B)
SCH_B = (127.0 - 0.06) * 128.0       # c=0.06 minimizes rms rel err (~1.8%)


def _chunks(total, step):
    out = []
    c = 0
    while c < total:
        out.append((c, min(step, total - c)))
        c += step
    return out


def build(NA, QC):
    """Build the per-core Bacc graph for padded anchor count NA."""
    assert NA % 128 == 0 and QC % 64 == 0 and QC <= NA
    T = NA // 128
    # attention query units (<= 512 wide); the ragged tail chunk stays last:
    # with the rolling carry its iterations overlap the previous chunk's
    # out-projection, and the final tail/out-proj ops are narrow (fast).
    QSPLIT = _chunks(QC, 512)
    # token chunks: a short first chunk lets the K/Q projections start
    # before LayerNorm (the prologue critical path) finishes later tiles.
    if NA > 256:
        CH = [(0, 256)] + [(c0 + 256, cw) for (c0, cw) in _chunks(NA - 256, 512)]
    else:
        CH = [(0, NA)]

    nc = bacc.Bacc("TRN2", target_bir_lowering=False, debug=False, num_devices=8)

    # x arrives host-packed as [128, T*D]: partition p holds tokens
    # p, 128+p, ... so each DMA line is T*1536 contiguous bytes per
    # partition (full HBM bandwidth; the [NA, D] layout only manages
    # 1536-byte lines).
    x_ext = nc.dram_tensor("x", [128, T * D], BF, kind="ExternalInput").ap()
    wq_ext = nc.dram_tensor("wq", [128, J * HL * 128], BF, kind="ExternalInput").ap()
    wk_ext = nc.dram_tensor("wk", [128, J * HL * 128], BF, kind="ExternalInput").ap()
    wv_ext = nc.dram_tensor("wv", [128, J * HL * 96], BF, kind="ExternalInput").ap()
    wo_ext = nc.dram_tensor("wo", [128, HL * D], BF, kind="ExternalInput").ap()
    bq_ext = nc.dram_tensor("bq", [128, HL], F32, kind="ExternalInput").ap()
    bk_ext = nc.dram_tensor("bk", [128, HL], F32, kind="ExternalInput").ap()
    bv_ext = nc.dram_tensor("bv", [HL * 96], F32, kind="ExternalInput").ap()
    km_ext = nc.dram_tensor("km", [1, NA], BF, kind="ExternalInput").ap()
    out_ext = nc.dram_tensor("out", [D, QC], BF, kind="ExternalOutput").ap()

    with tile.TileContext(nc) as tc:
        with (
            tc.tile_pool(name="singles", bufs=1) as singles,
            tc.tile_pool(name="work", bufs=5) as work,
            tc.tile_pool(name="probs", bufs=6) as probs_pool,
        ):
            # ---- x first (LN needs it immediately; queues are FIFO so
            # anything emitted before it would delay the whole prologue).
            # Two halves so LN can start after the first 4 tiles land.
            x_all = singles.tile([128, T, D], BF)
            x_v = x_ext.rearrange("p (t d) -> p t d", t=T)
            for (i0, iw) in _chunks(T, 2):
                nc.sync.dma_start(
                    out=x_all[:, i0:i0 + iw, :], in_=x_v[:, i0:i0 + iw, :])
            x_tiles = [x_all[:, i, :] for i in range(T)]

            # ---- weights / constants into SBUF (one DMA per tensor, issued
            # in consumption order: K first, Wo last).
            wq_sb = singles.tile([128, J, HL * 128], BF)
            wk_sb = singles.tile([128, J, HL * 128], BF)
            wv_sb = singles.tile([128, J, HL * 96], BF)
            wo_sb = singles.tile([128, HL, D], BF)
            nc.sync.dma_start(
                out=wk_sb, in_=wk_ext.rearrange("p (j e) -> p j e", j=J))
            nc.sync.dma_start(
                out=wq_sb, in_=wq_ext.rearrange("p (j e) -> p j e", j=J))
            nc.sync.dma_start(
                out=wv_sb, in_=wv_ext.rearrange("p (j e) -> p j e", j=J))
            nc.sync.dma_start(out=wo_sb, in_=wo_ext)
            bq_sb = singles.tile([128, HL], F32)
            nc.gpsimd.dma_start(out=bq_sb, in_=bq_ext)
            bk_sb = singles.tile([128, HL], F32)
            nc.gpsimd.dma_start(out=bk_sb, in_=bk_ext)
            bv_sb = singles.tile([128, HL * 96], F32)
            bv_bcast = bass.AP(
                tensor=bv_ext.tensor, offset=bv_ext.offset,
                ap=[[0, 128], [1, HL * 96]],
            )
            nc.gpsimd.dma_start(out=bv_sb, in_=bv_bcast)

            ones96 = singles.tile([1, 96], BF)
            nc.vector.memset(ones96, 1.0)
            eps_sb = singles.tile([128, 1], F32)
            nc.vector.memset(eps_sb, EPS)
            ident = singles.tile([128, 128], BF)
            from concourse.masks import make_identity
            make_identity(nc, ident)
            zT = [singles.tile([128, J, cw], BF, name=f"zT{c}")
                  for c, (c0, cw) in enumerate(CH)]

            def zt_slice(j, c0, cw):
                ci = max(i for i, (cc0, _) in enumerate(CH) if cc0 <= c0)
                off = c0 - CH[ci][0]
                assert off + cw <= CH[ci][1]
                return zT[ci][:, j, off:off + cw]

            kT = singles.tile([128, HL, NA], BF)
            qT = singles.tile([128, HL, QC], BF)
            v_sb = singles.tile([128, T, HL, 128], BF)
            # per-head tiles so the out-projection's per-head matmuls only
            # depend on their own head's tail (not the whole buffer)
            avn = [singles.tile([128, QC], BF, name=f"avn{h}")
                   for h in range(HL)]
            for h in range(HL):
                nc.gpsimd.memset(avn[h][96:128, :], 0.0)

            # v columns: 0..95 head dims, 96 ones (denominator), 97.. zero
            # (padding to 128 weights keeps FWL on for the av matmuls)
            nc.vector.memset(v_sb[:, :, :, 96:97], 1.0)
            nc.gpsimd.memset(v_sb[:, :, :, 97:128], 0.0)

            with (
                tc.tile_pool(name="ps_proj", bufs=2, space="PSUM") as ps_proj,
                tc.tile_pool(name="ps_t", bufs=3, space="PSUM") as ps_t,
            ):
                # Pipeline per 512-token chunk: LN/z -> transpose (on the
                # otherwise-idle PE) -> K/Q/V projections for that chunk.
                for ci, (c0, cw) in enumerate(CH):
                    tlo, thi = c0 // 128, (c0 + cw) // 128
                    for i in range(tlo, thi):
                        x_i = x_tiles[i]
                        x_g = x_i.rearrange("p (n f) -> p n f", f=384)
                        stats = work.tile([128, 2, 6], F32, tag="stats")
                        for g in range(2):
                            nc.vector.bn_stats(out=stats[:, g, :], in_=x_g[:, g, :])
                        mv = work.tile([128, 2], F32, tag="mv")
                        nc.vector.bn_aggr(out=mv, in_=stats)
                        sd = work.tile([128, 1], F32, tag="sd")
                        nc.scalar.activation(
                            out=sd, in_=mv[:, 1:2],
                            func=mybir.ActivationFunctionType.Sqrt,
                            bias=eps_sb, scale=1.0,
                        )
                        rstd = work.tile([128, 1], F32, tag="rstd")
                        nc.vector.reciprocal(out=rstd, in_=sd)
                        z_i = work.tile([128, D], BF, tag="z")
                        nc.vector.tensor_scalar(
                            out=z_i, in0=x_i,
                            scalar1=mv[:, 0:1], scalar2=rstd,
                            op0=mybir.AluOpType.subtract, op1=mybir.AluOpType.mult,
                        )
                        ioff = (i - tlo) * 128
                        tp = ps_t.tile([128, J, 128], BF, tag="tp")
                        for j in range(J):
                            nc.tensor.transpose(
                                tp[:, j, :], z_i[:, j * 128:(j + 1) * 128],
                                ident)
                        nc.scalar.activation(
                            out=zT[ci][:, :, ioff:ioff + 128], in_=tp,
                            func=mybir.ActivationFunctionType.Copy,
                        )

                    # K / Q projections for this chunk (local heads).
                    # Q only covers [0, QC) — queries past the last anchor
                    # are never read.
                    qw_c = min(cw, max(0, QC - c0))
                    for (name, w_sb, b_sb, dst, ncols) in (
                        ("k", wk_sb, bk_sb, kT, cw),
                        ("q", wq_sb, bq_sb, qT, qw_c),
                    ):
                        if ncols == 0:
                            continue
                        for m in range(HL):
                            ps = ps_proj.tile([128, ncols], F32, tag="proj")
                            for j in range(J):
                                nc.tensor.matmul(
                                    ps,
                                    lhsT=w_sb[:, j, m * 128:(m + 1) * 128],
                                    rhs=zT[ci][:, j, :ncols],
                                    start=(j == 0), stop=(j == J - 1),
                                )
                            nc.vector.tensor_scalar_add(
                                out=dst[:, m, c0:c0 + ncols], in0=ps,
                                scalar1=b_sb[:, m:m + 1],
                            )
                    # V projection for this chunk's token tiles (all 4 local
                    # heads in one N=384 matmul per contraction block)
                    for i in range(tlo, thi):
                        ps = ps_proj.tile([128, HL * 96], F32, tag="proj")
                        for j in range(J):
                            nc.tensor.matmul(
                                ps,
                                lhsT=zt_slice(j, i * 128, 128),
                                rhs=wv_sb[:, j, :],
                                start=(j == 0), stop=(j == J - 1),
                            )
                        nc.vector.tensor_tensor(
                            out=v_sb[:, i, :, 0:96],
                            in0=ps.rearrange("p (h c) -> p h c", c=96),
                            in1=bv_sb.rearrange("p (h c) -> p h c", c=96),
                            op=mybir.AluOpType.add,
                        )

                # overwrite kT row 96 of every head with the key-pad mask row
                km_bcast = bass.AP(
                    tensor=km_ext.tensor, offset=km_ext.offset,
                    ap=[[0, 1], [0, HL], [1, NA]],
                )
                nc.gpsimd.dma_start(out=kT[96:97, :, :], in_=km_bcast)

            # ---- attention + out-projection, fused per query chunk.
            # Per (query-chunk, head): scores for 2 key-tiles land in a
            # 2-bank PSUM group; ONE 1024-wide exp per group (halves the
            # per-instruction overhead on ScalarE — the phase bottleneck).
            # av matmuls for group g are emitted right after exp g, so the
            # in-order TensorE executes them while exp g+1 runs. After all
            # heads of a chunk, that chunk's out-projection is emitted; its
            # matmuls + output DMA overlap the next chunk's attention.
            # PSUM budget: scores 2x2 + av 2 + out-proj 2 = 8 banks.
            TKG = _chunks(T, 2)

            with (
                tc.tile_pool(name="ps_s", bufs=2, space="PSUM") as ps_s,
                tc.tile_pool(name="ps_av", bufs=2, space="PSUM") as ps_av,
                tc.tile_pool(name="ps_o", bufs=2, space="PSUM") as ps_o,
            ):
                def emit_av(h, grp, av_ps, qw):
                    t0, tn, pb = grp
                    for gi in range(tn):
                        tk = t0 + gi
                        nc.tensor.matmul(
                            av_ps[:, :qw],
                            lhsT=v_sb[:, tk, h, :],
                            rhs=pb[:, gi, :qw],
                            start=(tk == 0), stop=(tk == T - 1),
                            skip_group_check=True,
                        )

                def tail(h, q0, qw, av_ps):
                    # normalize: avn = avT[0:96] * (1 / avT[96]) broadcast.
                    # (recip_approx_fast is a bitwise-seed custom op — needs
                    # its input in SBUF, so copy the denominator row first.)
                    d_sb = work.tile([1, qw], F32, tag="dsb")
                    nc.vector.tensor_copy(out=d_sb, in_=av_ps[96:97, :qw])
                    rec32 = work.tile([1, qw], F32, tag="rec32")
                    nc.vector.reciprocal_approx_fast(out=rec32, in_=d_sb)
                    recip_bf = work.tile([1, qw], BF, tag="recipbf")
                    nc.vector.tensor_copy(out=recip_bf, in_=rec32)
                    bc_sb = work.tile([96, qw], BF, tag="bc")
                    nc.gpsimd.partition_broadcast(out_ap=bc_sb, in_ap=recip_bf)
                    nc.vector.tensor_tensor(
                        out=avn[h][0:96, q0:q0 + qw],
                        in0=av_ps[0:96, :qw], in1=bc_sb,
                        op=mybir.AluOpType.mult,
                    )

                # exp groups handled by DVE (Schraudolph bf16 bit-trick,
                # ~1.8% rms) instead of ScalarE's exact LUT exp: every 3rd
                # group starting at 1 (so ~1/3 of the exp stream moves off
                # the ScalarE bottleneck; error contribution ~1%).
                ngrp = len(TKG)
                dve_groups = {g for g in range(1, ngrp, 3)} if ngrp >= 3 else set()

                def emit_outproj(q0, qw):
                    # partial out projection for one query chunk (sum over
                    # local heads; host adds the pair partials + bias)
                    for m in range(J):
                        o_ps = ps_o.tile([128, 512], F32, tag="o")
                        for hh in range(HL):
                            nc.tensor.matmul(
                                o_ps[:, :qw],
                                lhsT=wo_sb[:, hh, m * 128:(m + 1) * 128],
                                rhs=avn[hh][:, q0:q0 + qw],
                                start=(hh == 0), stop=(hh == HL - 1),
                            )
                        o_sb = work.tile([128, 512], BF, tag="osb")
                        # ScalarE evacuates (DVE is loaded with Schraudolph
                        # exps + tails in this phase)
                        nc.scalar.activation(
                            out=o_sb[:, :qw], in_=o_ps[:, :qw],
                            func=mybir.ActivationFunctionType.Copy,
                        )
                        eng = nc.sync if (m % 2 == 0) else nc.gpsimd
                        eng.dma_start(
                            out=out_ext[m * 128:(m + 1) * 128, q0:q0 + qw],
                            in_=o_sb[:, :qw],
                        )

                # Rolling software pipeline across (query-chunk, head)
                # iterations: iteration i's last av group + tail (and, after
                # a chunk's final head, that chunk's out-projection) are
                # emitted inside iteration i+1, after its first exp — so the
                # in-order TensorE always has the next iteration's scores to
                # chew on while ScalarE/VectorE finish iteration i.
                iters = [(q0, qw, h) for (q0, qw) in QSPLIT for h in range(HL)]
                carry = None          # (h, pend_grp, av_ps, q0, qw, oproj)
                for (q0, qw, h) in iters:
                    av_ps = ps_av.tile([128, 512], F32, tag="av")
                    pend = None
                    for g, (t0, tn) in enumerate(TKG):
                        s_ps = ps_s.tile([128, 2, 512], F32, tag="s")
                        for gi in range(tn):
                            tk = t0 + gi
                            nc.tensor.matmul(
                                s_ps[:, gi, :qw],
                                lhsT=kT[:, h, tk * 128:(tk + 1) * 128],
                                rhs=qT[:, h, q0:q0 + qw],
                                start=True, stop=True,
                            )
                        pb = probs_pool.tile([128, 2, 512], BF, tag="p")
                        if g in dve_groups:
                            nc.vector.tensor_scalar(
                                out=pb.bitcast(mybir.dt.int16)[:, :tn, :qw],
                                in0=s_ps[:, :tn, :qw],
                                scalar1=float(SCH_A), scalar2=float(SCH_B),
                                op0=mybir.AluOpType.mult,
                                op1=mybir.AluOpType.add,
                            )
                        else:
                            nc.scalar.activation(
                                out=pb[:, :tn, :qw], in_=s_ps[:, :tn, :qw],
                                func=mybir.ActivationFunctionType.Exp,
                                scale=float(SCALE),
                            )
                        if g == 0 and carry is not None:
                            ch_, cpend, cav, cq0, cqw, coproj = carry
                            emit_av(ch_, cpend, cav, cqw)
                            tail(ch_, cq0, cqw, cav)
                            if coproj:
                                emit_outproj(cq0, cqw)
                            carry = None
                        if pend is not None:
                            emit_av(h, pend, av_ps, qw)
                        pend = (t0, tn, pb)
                    carry = (h, pend, av_ps, q0, qw, h == HL - 1)
                ch_, cpend, cav, cq0, cqw, coproj = carry
                emit_av(ch_, cpend, cav, cqw)
                tail(ch_, cq0, cqw, cav)
                if coproj:
                    emit_outproj(cq0, cqw)

    nc.compile()
    return nc


_CACHE = {}


def _prep_weights(ln_g, ln_b, Wq, bq, Wk, bk, Wv, bv, Wo, bo):
    """Per-head-group device weight layouts. Returns [group0, group1]."""
    def pad_head_T(W, hg):
        # (W * g).T for heads of the group, padded 96 -> 128 cols, then
        # SBUF layout (128, J, HL*128): [p, j, e] = WT[j*128+p, e]
        WT = (W * ln_g[None, :]).T.astype(np.float32)
        WT = WT.reshape(D, H, 96)[:, hg * HL:(hg + 1) * HL, :]
        Wp = np.zeros((D, HL, 128), np.float32)
        Wp[:, :, :96] = WT
        Wp = Wp.reshape(J, 128, HL * 128).transpose(1, 0, 2)
        return np.ascontiguousarray(Wp.reshape(128, J * HL * 128)).astype(BF16)

    def plain_T(W, hg):
        WT = (W * ln_g[None, :]).T.astype(np.float32)
        WT = WT.reshape(D, H, 96)[:, hg * HL:(hg + 1) * HL, :].reshape(D, HL * 96)
        Wp = WT.reshape(J, 128, HL * 96).transpose(1, 0, 2)
        return np.ascontiguousarray(Wp.reshape(128, J * HL * 96)).astype(BF16)

    def pad_bias(bb, hg, ones_row):
        bp = np.zeros((HL, 128), np.float32)
        bp[:, :96] = bb.reshape(H, 96)[hg * HL:(hg + 1) * HL]
        if ones_row:
            bp[:, 96] = 1.0
        return np.ascontiguousarray(bp.T).astype(np.float32)  # (128, HL)

    def pad_wo(hg):
        w = np.zeros((128, HL, D), np.float32)
        w[:96] = Wo.T.reshape(H, 96, D)[hg * HL:(hg + 1) * HL].transpose(1, 0, 2)
        return np.ascontiguousarray(w.reshape(128, HL * D)).astype(BF16)

    bbq = Wq @ ln_b + bq
    bbk = Wk @ ln_b + bk
    bbv = Wv @ ln_b + bv
    return [{
        "wq": pad_head_T(Wq, hg),
        "wk": pad_head_T(Wk, hg),
        "wv": plain_T(Wv, hg),
        "wo": pad_wo(hg),
        "bq": pad_bias(bbq, hg, True),
        "bk": pad_bias(bbk, hg, False),
        "bv": np.ascontiguousarray(
            bbv.reshape(H, 96)[hg * HL:(hg + 1) * HL].reshape(-1)
        ).astype(np.float32),
    } for hg in range(2)]


def _make_in_maps(hidden_states, idx, NA, wmaps):
    in_maps = []
    for c in range(8):
        b, hg = c // 2, c % 2
        nb = len(idx[b])
        xg = np.zeros((NA, D), np.float32)
        xg[:nb] = hidden_states[b][idx[b]]
        # pack [NA, D] -> [128, T*D]: partition p gets tokens p, 128+p, ...
        T = NA // 128
        xp = np.ascontiguousarray(
            xg.reshape(T, 128, D).transpose(1, 0, 2).reshape(128, T * D))
        km = np.zeros((NA,), np.float32)
        km[nb:] = MASK_NEG
        in_maps.append({
            "x": xp.astype(BF16),
            "km": km.reshape(1, NA).astype(BF16),
            **wmaps[hg],
        })
    return in_maps


def kernel(hidden_states, anchor_mask, ln_g, ln_b,
           Wq, bq, Wk, bk, Wv, bv, Wo, bo):
    hidden_states = np.asarray(hidden_states, dtype=np.float32)
    anchor_mask = np.asarray(anchor_mask).astype(bool)
    args = [np.asarray(a, dtype=np.float32)
            for a in (ln_g, ln_b, Wq, bq, Wk, bk, Wv, bv, Wo, bo)]
    bo_f = args[-1]

    idx = [np.nonzero(anchor_mask[b])[0] for b in range(B)]
    max_nb = max(len(i) for i in idx)
    NA = max(256, ((max_nb + 127) // 128) * 128)
    QC = max(128, ((max_nb + 63) // 64) * 64)

    if (NA, QC) not in _CACHE:
        _CACHE[(NA, QC)] = build(NA, QC)
    nc = _CACHE[(NA, QC)]

    wmaps = _prep_weights(*args)
    in_maps = _make_in_maps(hidden_states, idx, NA, wmaps)

    res = run_bass_kernel_spmd(nc, in_maps, core_ids=list(range(8)))

    out = np.zeros((B, S, D), np.float32)
    for b in range(B):
        nb = len(idx[b])
        oT = (res.results[2 * b]["out"].astype(np.float32)
              + res.results[2 * b + 1]["out"].astype(np.float32))
        out[b, idx[b]] = oT.T[:nb] + bo_f[None, :]
    return out



# revision 29
# speedup vs baseline: 1.0404x; 1.0016x over previous
"""AnchorAttention Trainium2 kernel (8 NeuronCores, SPMD, no collectives).

Math (per batch): gather anchor rows of hidden_states, LayerNorm, QKV
projections, dense attention among anchors only, out-projection, scatter
back (non-anchor rows of the output are zero; keys are anchors only).

Sharding: core c handles batch c//2 and HEAD GROUP c%2 (4 of 8 heads).
Both cores of a pair see the same gathered anchor tokens; each computes
q/k/v and attention for its 4 heads over ALL anchors, then a partial
out-projection (sum over its heads). The host adds the two partials
(+ output bias) — out-projection is linear in heads, so no collective
is needed.

Device layout (contraction dims on partitions):
  zT   per 512-token chunk: (128, 6, 512)  z = (x-mu)*rstd, d on partitions
  qT   (128, 4, NA)  per head 128 rows: 96 hd + row96 == 1.0 (mask helper)
  kT   (128, 4, NA)  per head 128 rows: 96 hd + row96 == key-pad mask
  v    (128, T, 4, 128) plain layout: 96 head dims + ones col + zero pad
  scores^T (tk, tq) per (head, 2-tile key group, query-chunk); one
  1024-wide exp per group; probs = exp(scale*s)
  avT  (128, NQH) accumulated over tk; row 96 = softmax denominator
  outT (768, NA) = sum_h Wo_h^T @ (avT_h / denom_h)   [bias added on host]

LayerNorm's affine (ln_g, ln_b) is folded into the weights on the host:
W~ = W * g, bias~ = W @ b + bias.
"""

import numpy as np
import ml_dtypes

import concourse.bass as bass
import concourse.mybir as mybir
import concourse.tile as tile
from concourse import bacc
from concourse.bass_utils import run_bass_kernel_spmd

BF16 = ml_dtypes.bfloat16
F32 = mybir.dt.float32
BF = mybir.dt.bfloat16

B, S, D, H, HD = 4, 2048, 768, 8, 96
HL = H // 2           # heads per core
J = D // 128          # contraction blocks
EPS = 1e-5
SCALE = 1.0 / np.sqrt(HD)
MASK_NEG = -800.0     # exp(SCALE*(qk+MASK_NEG)) ~ 4e-36 (and keeps the
                      # Schraudolph int16 path positive / unsaturated)
LOG2E = 1.4426950408889634
SCH_A = SCALE * 128.0 * LOG2E        # bf16-bits = round(score*SCH_A + SCH_B)
SCH_B = (127.0 - 0.06) * 128.0       # c=0.06 minimizes rms rel err (~1.8%)


def _chunks(total, step):
    out = []
    c = 0
    while c < total:
        out.append((c, min(step, total - c)))
        c += step
    return out


def build(NA, QC):
    """Build the per-core Bacc graph for padded anchor count NA."""
    assert NA % 128 == 0 and QC % 64 == 0 and QC <= NA
    T = NA // 128
    # attention query units (<= 512 wide); the ragged tail chunk stays last:
    # with the rolling carry its iterations overlap the previous chunk's
    # out-projection, and the final tail/out-proj ops are narrow (fast).
    QSPLIT = _chunks(QC, 512)
    # token chunks: a short first chunk lets the K/Q projections start
    # before LayerNorm (the prologue critical path) finishes later tiles.
    if NA > 256:
        CH = [(0, 256)] + [(c0 + 256, cw) for (c0, cw) in _chunks(NA - 256, 512)]
    else:
        CH = [(0, NA)]

    nc = bacc.Bacc("TRN2", target_bir_lowering=False, debug=False, num_devices=8)

    # x arrives host-packed as [128, T*D]: partition p holds tokens
    # p, 128+p, ... so each DMA line is T*1536 contiguous bytes per
    # partition (full HBM bandwidth; the [NA, D] layout only manages
    # 1536-byte lines).
    x_ext = nc.dram_tensor("x", [128, T * D], BF, kind="ExternalInput").ap()
    wq_ext = nc.dram_tensor("wq", [128, J * HL * 128], BF, kind="ExternalInput").ap()
    wk_ext = nc.dram_tensor("wk", [128, J * HL * 128], BF, kind="ExternalInput").ap()
    wv_ext = nc.dram_tensor("wv", [128, J * HL * 96], BF, kind="ExternalInput").ap()
    wo_ext = nc.dram_tensor("wo", [128, HL * D], BF, kind="ExternalInput").ap()
    bq_ext = nc.dram_tensor("bq", [128, HL], F32, kind="ExternalInput").ap()
    bk_ext = nc.dram_tensor("bk", [128, HL], F32, kind="ExternalInput").ap()
    bv_ext = nc.dram_tensor("bv", [HL * 96], F32, kind="ExternalInput").ap()
    km_ext = nc.dram_tensor("km", [1, NA], BF, kind="ExternalInput").ap()
    out_ext = nc.dram_tensor("out", [D, QC], BF, kind="ExternalOutput").ap()

    with tile.TileContext(nc) as tc:
        with (
            tc.tile_pool(name="singles", bufs=1) as singles,
            tc.tile_pool(name="work", bufs=5) as work,
            tc.tile_pool(name="probs", bufs=6) as probs_pool,
        ):
            # ---- x first (LN needs it immediately; queues are FIFO so
            # anything emitted before it would delay the whole prologue).
            # Two halves so LN can start after the first 4 tiles land.
            x_all = singles.tile([128, T, D], BF)
            x_v = x_ext.rearrange("p (t d) -> p t d", t=T)
            nc.sync.dma_start(out=x_all[:, 0:1, :], in_=x_v[:, 0:1, :])
            for (i0, iw) in _chunks(T - 1, 2):
                nc.sync.dma_start(
                    out=x_all[:, 1 + i0:1 + i0 + iw, :],
                    in_=x_v[:, 1 + i0:1 + i0 + iw, :])
            x_tiles = [x_all[:, i, :] for i in range(T)]

            # ---- weights / constants into SBUF (one DMA per tensor, issued
            # in consumption order: K first, Wo last).
            wq_sb = singles.tile([128, J, HL * 128], BF)
            wk_sb = singles.tile([128, J, HL * 128], BF)
            wv_sb = singles.tile([128, J, HL * 96], BF)
            wo_sb = singles.tile([128, HL, D], BF)
            nc.sync.dma_start(
                out=wk_sb, in_=wk_ext.rearrange("p (j e) -> p j e", j=J))
            nc.sync.dma_start(
                out=wq_sb, in_=wq_ext.rearrange("p (j e) -> p j e", j=J))
            nc.sync.dma_start(
                out=wv_sb, in_=wv_ext.rearrange("p (j e) -> p j e", j=J))
            nc.sync.dma_start(out=wo_sb, in_=wo_ext)
            bq_sb = singles.tile([128, HL], F32)
            nc.gpsimd.dma_start(out=bq_sb, in_=bq_ext)
            bk_sb = singles.tile([128, HL], F32)
            nc.gpsimd.dma_start(out=bk_sb, in_=bk_ext)
            bv_sb = singles.tile([128, HL * 96], F32)
            bv_bcast = bass.AP(
                tensor=bv_ext.tensor, offset=bv_ext.offset,
                ap=[[0, 128], [1, HL * 96]],
            )
            nc.gpsimd.dma_start(out=bv_sb, in_=bv_bcast)

            ones96 = singles.tile([1, 96], BF)
            nc.vector.memset(ones96, 1.0)
            eps_sb = singles.tile([128, 1], F32)
            nc.vector.memset(eps_sb, EPS)
            ident = singles.tile([128, 128], BF)
            from concourse.masks import make_identity
            make_identity(nc, ident)

            # HAM warmup: ~45 small matmuls on the identity while the x DMA
            # and LayerNorm are still in flight. The PE clock-gate needs
            # ~3.4us of sustained matmul activity to release 2.4 GHz (and
            # transpose-mode doesn't count) — without this the first real
            # projection matmuls run at 1.2 GHz.
            with tc.tile_pool(name="ps_w", bufs=1, space="PSUM") as ps_w:
                warm_ps = ps_w.tile([128, 128], F32)
                for _ in range(45):
                    nc.tensor.matmul(
                        warm_ps, lhsT=ident, rhs=ident, start=True, stop=True)
            zT = [singles.tile([128, J, cw], BF, name=f"zT{c}")
                  for c, (c0, cw) in enumerate(CH)]

            def zt_slice(j, c0, cw):
                ci = max(i for i, (cc0, _) in enumerate(CH) if cc0 <= c0)
                off = c0 - CH[ci][0]
                assert off + cw <= CH[ci][1]
                return zT[ci][:, j, off:off + cw]

            kT = singles.tile([128, HL, NA], BF)
            qT = singles.tile([128, HL, QC], BF)
            v_sb = singles.tile([128, T, HL, 128], BF)
            # per-head tiles so the out-projection's per-head matmuls only
            # depend on their own head's tail (not the whole buffer)
            avn = [singles.tile([128, QC], BF, name=f"avn{h}")
                   for h in range(HL)]
            for h in range(HL):
                nc.gpsimd.memset(avn[h][96:128, :], 0.0)

            # v columns: 0..95 head dims, 96 ones (denominator), 97.. zero
            # (padding to 128 weights keeps FWL on for the av matmuls)
            nc.vector.memset(v_sb[:, :, :, 96:97], 1.0)
            nc.gpsimd.memset(v_sb[:, :, :, 97:128], 0.0)

            with (
                tc.tile_pool(name="ps_proj", bufs=2, space="PSUM") as ps_proj,
                tc.tile_pool(name="ps_t", bufs=3, space="PSUM") as ps_t,
            ):
                # Pipeline per 512-token chunk: LN/z -> transpose (on the
                # otherwise-idle PE) -> K/Q/V projections for that chunk.
                for ci, (c0, cw) in enumerate(CH):
                    tlo, thi = c0 // 128, (c0 + cw) // 128
                    for i in range(tlo, thi):
                        x_i = x_tiles[i]
                        x_g = x_i.rearrange("p (n f) -> p n f", f=384)
                        stats = work.tile([128, 2, 6], F32, tag="stats")
                        for g in range(2):
                            nc.vector.bn_stats(out=stats[:, g, :], in_=x_g[:, g, :])
                        mv = work.tile([128, 2], F32, tag="mv")
                        nc.vector.bn_aggr(out=mv, in_=stats)
                        sd = work.tile([128, 1], F32, tag="sd")
                        nc.scalar.activation(
                            out=sd, in_=mv[:, 1:2],
                            func=mybir.ActivationFunctionType.Sqrt,
                            bias=eps_sb, scale=1.0,
                        )
                        rstd = work.tile([128, 1], F32, tag="rstd")
                        nc.vector.reciprocal(out=rstd, in_=sd)
                        z_i = work.tile([128, D], BF, tag="z")
                        nc.vector.tensor_scalar(
                            out=z_i, in0=x_i,
                            scalar1=mv[:, 0:1], scalar2=rstd,
                            op0=mybir.AluOpType.subtract, op1=mybir.AluOpType.mult,
                        )
                        ioff = (i - tlo) * 128
                        tp = ps_t.tile([128, J, 128], BF, tag="tp")
                        for j in range(J):
                            nc.tensor.transpose(
                                tp[:, j, :], z_i[:, j * 128:(j + 1) * 128],
                                ident)
                        nc.scalar.activation(
                            out=zT[ci][:, :, ioff:ioff + 128], in_=tp,
                            func=mybir.ActivationFunctionType.Copy,
                        )

                    # K / Q projections for this chunk (local heads).
                    # Q only covers [0, QC) — queries past the last anchor
                    # are never read.
                    qw_c = min(cw, max(0, QC - c0))
                    for (name, w_sb, b_sb, dst, ncols) in (
                        ("k", wk_sb, bk_sb, kT, cw),
                        ("q", wq_sb, bq_sb, qT, qw_c),
                    ):
                        if ncols == 0:
                            continue
                        for m in range(HL):
                            ps = ps_proj.tile([128, ncols], F32, tag="proj")
                            for j in range(J):
                                nc.tensor.matmul(
                                    ps,
                                    lhsT=w_sb[:, j, m * 128:(m + 1) * 128],
                                    rhs=zT[ci][:, j, :ncols],
                                    start=(j == 0), stop=(j == J - 1),
                                )
                            nc.vector.tensor_scalar_add(
                                out=dst[:, m, c0:c0 + ncols], in0=ps,
                                scalar1=b_sb[:, m:m + 1],
                            )
                    # V projection for this chunk's token tiles (all 4 local
                    # heads in one N=384 matmul per contraction block)
                    for i in range(tlo, thi):
                        ps = ps_proj.tile([128, HL * 96], F32, tag="proj")
                        for j in range(J):
                            nc.tensor.matmul(
                                ps,
                                lhsT=zt_slice(j, i * 128, 128),
                                rhs=wv_sb[:, j, :],
                                start=(j == 0), stop=(j == J - 1),
                            )
                        nc.vector.tensor_tensor(
                            out=v_sb[:, i, :, 0:96],
                            in0=ps.rearrange("p (h c) -> p h c", c=96),
                            in1=bv_sb.rearrange("p (h c) -> p h c", c=96),
                            op=mybir.AluOpType.add,
                        )

                # overwrite kT row 96 of every head with the key-pad mask row
                km_bcast = bass.AP(
                    tensor=km_ext.tensor, offset=km_ext.offset,
                    ap=[[0, 1], [0, HL], [1, NA]],
                )
                nc.gpsimd.dma_start(out=kT[96:97, :, :], in_=km_bcast)

            # ---- attention + out-projection, fused per query chunk.
            # Per (query-chunk, head): scores for 2 key-tiles land in a
            # 2-bank PSUM group; ONE 1024-wide exp per group (halves the
            # per-instruction overhead on ScalarE — the phase bottleneck).
            # av matmuls for group g are emitted right after exp g, so the
            # in-order TensorE executes them while exp g+1 runs. After all
            # heads of a chunk, that chunk's out-projection is emitted; its
            # matmuls + output DMA overlap the next chunk's attention.
            # PSUM budget: scores 2x2 + av 2 + out-proj 2 = 8 banks.
            TKG = _chunks(T, 2)

            with (
                tc.tile_pool(name="ps_s", bufs=2, space="PSUM") as ps_s,
                tc.tile_pool(name="ps_av", bufs=2, space="PSUM") as ps_av,
                tc.tile_pool(name="ps_o", bufs=2, space="PSUM") as ps_o,
            ):
                def emit_av(h, grp, av_ps, qw):
                    t0, tn, pb = grp
                    for gi in range(tn):
                        tk = t0 + gi
                        nc.tensor.matmul(
                            av_ps[:, :qw],
                            lhsT=v_sb[:, tk, h, :],
                            rhs=pb[:, gi, :qw],
                            start=(tk == 0), stop=(tk == T - 1),
                            skip_group_check=True,
                        )

                def tail(h, q0, qw, av_ps):
                    # normalize: avn = avT[0:96] * (1 / avT[96]) broadcast.
                    # (recip_approx_fast is a bitwise-seed custom op — needs
                    # its input in SBUF, so copy the denominator row first.)
                    d_sb = work.tile([1, qw], F32, tag="dsb")
                    nc.vector.tensor_copy(out=d_sb, in_=av_ps[96:97, :qw])
                    rec32 = work.tile([1, qw], F32, tag="rec32")
                    nc.vector.reciprocal_approx_fast(out=rec32, in_=d_sb)
                    recip_bf = work.tile([1, qw], BF, tag="recipbf")
                    nc.vector.tensor_copy(out=recip_bf, in_=rec32)
                    bc_sb = work.tile([96, qw], BF, tag="bc")
                    nc.gpsimd.partition_broadcast(out_ap=bc_sb, in_ap=recip_bf)
                    nc.vector.tensor_tensor(
                        out=avn[h][0:96, q0:q0 + qw],
                        in0=av_ps[0:96, :qw], in1=bc_sb,
                        op=mybir.AluOpType.mult,
                    )

                # exp groups handled by DVE (Schraudolph bf16 bit-trick,
                # ~1.8% rms) instead of ScalarE's exact LUT exp: every 3rd
                # group starting at 1 (so ~1/3 of the exp stream moves off
                # the ScalarE bottleneck; error contribution ~1%).
                # never the last group: its exp gates the carry flush (av +
                # tail) and the DVE queue is backlogged at that point
                ngrp = len(TKG)
                if ngrp >= 5:
                    dve_groups = {g for g in range(1, ngrp - 1, 2)}
                elif ngrp >= 3:
                    dve_groups = {1}
                else:
                    dve_groups = set()

                def emit_outproj(q0, qw):
                    # partial out projection for one query chunk (sum over
                    # local heads; host adds the pair partials + bias)
                    for m in range(J):
                        o_ps = ps_o.tile([128, 512], F32, tag="o")
                        for hh in range(HL):
                            nc.tensor.matmul(
                                o_ps[:, :qw],
                                lhsT=wo_sb[:, hh, m * 128:(m + 1) * 128],
                                rhs=avn[hh][:, q0:q0 + qw],
                                start=(hh == 0), stop=(hh == HL - 1),
                            )
                        o_sb = work.tile([128, 512], BF, tag="osb")
                        # ScalarE evacuates (DVE is loaded with Schraudolph
                        # exps + tails in this phase)
                        nc.scalar.activation(
                            out=o_sb[:, :qw], in_=o_ps[:, :qw],
                            func=mybir.ActivationFunctionType.Copy,
                        )
                        eng = nc.sync if (m % 2 == 0) else nc.gpsimd
                        eng.dma_start(
                            out=out_ext[m * 128:(m + 1) * 128, q0:q0 + qw],
                            in_=o_sb[:, :qw],
                        )

                # Rolling software pipeline across (query-chunk, head)
                # iterations: iteration i's last av group + tail (and, after
                # a chunk's final head, that chunk's out-projection) are
                # emitted inside iteration i+1, after its first exp — so the
                # in-order TensorE always has the next iteration's scores to
                # chew on while ScalarE/VectorE finish iteration i.
                iters = [(q0, qw, h) for (q0, qw) in QSPLIT for h in range(HL)]
                carry = None          # (h, pend_grp, av_ps, q0, qw, oproj)
                for (q0, qw, h) in iters:
                    av_ps = ps_av.tile([128, 512], F32, tag="av")
                    pend = None
                    for g, (t0, tn) in enumerate(TKG):
                        s_ps = ps_s.tile([128, 2, 512], F32, tag="s")
                        for gi in range(tn):
                            tk = t0 + gi
                            nc.tensor.matmul(
                                s_ps[:, gi, :qw],
                                lhsT=kT[:, h, tk * 128:(tk + 1) * 128],
                                rhs=qT[:, h, q0:q0 + qw],
                                start=True, stop=True,
                            )
                        pb = probs_pool.tile([128, 2, 512], BF, tag="p")
                        if g in dve_groups:
                            nc.vector.tensor_scalar(
                                out=pb.bitcast(mybir.dt.int16)[:, :tn, :qw],
                                in0=s_ps[:, :tn, :qw],
                                scalar1=float(SCH_A), scalar2=float(SCH_B),
                                op0=mybir.AluOpType.mult,
                                op1=mybir.AluOpType.add,
                            )
                        else:
                            nc.scalar.activation(
                                out=pb[:, :tn, :qw], in_=s_ps[:, :tn, :qw],
                                func=mybir.ActivationFunctionType.Exp,
                                scale=float(SCALE),
                            )
                        if g == 0 and carry is not None:
                            ch_, cpend, cav, cq0, cqw, coproj = carry
                            emit_av(ch_, cpend, cav, cqw)
                            tail(ch_, cq0, cqw, cav)
                            if coproj:
                                emit_outproj(cq0, cqw)
                            carry = None
                        if pend is not None:
                            emit_av(h, pend, av_ps, qw)
                        pend = (t0, tn, pb)
                    carry = (h, pend, av_ps, q0, qw, h == HL - 1)
                ch_, cpend, cav, cq0, cqw, coproj = carry
                emit_av(ch_, cpend, cav, cqw)
                tail(ch_, cq0, cqw, cav)
                if coproj:
                    emit_outproj(cq0, cqw)

    nc.compile()
    return nc


_CACHE = {}


def _prep_weights(ln_g, ln_b, Wq, bq, Wk, bk, Wv, bv, Wo, bo):
    """Per-head-group device weight layouts. Returns [group0, group1]."""
    def pad_head_T(W, hg):
        # (W * g).T for heads of the group, padded 96 -> 128 cols, then
        # SBUF layout (128, J, HL*128): [p, j, e] = WT[j*128+p, e]
        WT = (W * ln_g[None, :]).T.astype(np.float32)
        WT = WT.reshape(D, H, 96)[:, hg * HL:(hg + 1) * HL, :]
        Wp = np.zeros((D, HL, 128), np.float32)
        Wp[:, :, :96] = WT
        Wp = Wp.reshape(J, 128, HL * 128).transpose(1, 0, 2)
        return np.ascontiguousarray(Wp.reshape(128, J * HL * 128)).astype(BF16)

    def plain_T(W, hg):
        WT = (W * ln_g[None, :]).T.astype(np.float32)
        WT = WT.reshape(D, H, 96)[:, hg * HL:(hg + 1) * HL, :].reshape(D, HL * 96)
        Wp = WT.reshape(J, 128, HL * 96).transpose(1, 0, 2)
        return np.ascontiguousarray(Wp.reshape(128, J * HL * 96)).astype(BF16)

    def pad_bias(bb, hg, ones_row):
        bp = np.zeros((HL, 128), np.float32)
        bp[:, :96] = bb.reshape(H, 96)[hg * HL:(hg + 1) * HL]
        if ones_row:
            bp[:, 96] = 1.0
        return np.ascontiguousarray(bp.T).astype(np.float32)  # (128, HL)

    def pad_wo(hg):
        w = np.zeros((128, HL, D), np.float32)
        w[:96] = Wo.T.reshape(H, 96, D)[hg * HL:(hg + 1) * HL].transpose(1, 0, 2)
        return np.ascontiguousarray(w.reshape(128, HL * D)).astype(BF16)

    bbq = Wq @ ln_b + bq
    bbk = Wk @ ln_b + bk
    bbv = Wv @ ln_b + bv
    return [{
        "wq": pad_head_T(Wq, hg),
        "wk": pad_head_T(Wk, hg),
        "wv": plain_T(Wv, hg),
        "wo": pad_wo(hg),
        "bq": pad_bias(bbq, hg, True),
        "bk": pad_bias(bbk, hg, False),
        "bv": np.ascontiguousarray(
            bbv.reshape(H, 96)[hg * HL:(hg + 1) * HL].reshape(-1)
        ).astype(np.float32),
    } for hg in range(2)]


def _make_in_maps(hidden_states, idx, NA, wmaps):
    in_maps = []
    for c in range(8):
        b, hg = c // 2, c % 2
        nb = len(idx[b])
        xg = np.zeros((NA, D), np.float32)
        xg[:nb] = hidden_states[b][idx[b]]
        # pack [NA, D] -> [128, T*D]: partition p gets tokens p, 128+p, ...
        T = NA // 128
        xp = np.ascontiguousarray(
            xg.reshape(T, 128, D).transpose(1, 0, 2).reshape(128, T * D))
        km = np.zeros((NA,), np.float32)
        km[nb:] = MASK_NEG
        in_maps.append({
            "x": xp.astype(BF16),
            "km": km.reshape(1, NA).astype(BF16),
            **wmaps[hg],
        })
    return in_maps


def kernel(hidden_states, anchor_mask, ln_g, ln_b,
           Wq, bq, Wk, bk, Wv, bv, Wo, bo):
    hidden_states = np.asarray(hidden_states, dtype=np.float32)
    anchor_mask = np.asarray(anchor_mask).astype(bool)
    args = [np.asarray(a, dtype=np.float32)
            for a in (ln_g, ln_b, Wq, bq, Wk, bk, Wv, bv, Wo, bo)]
    bo_f = args[-1]

    idx = [np.nonzero(anchor_mask[b])[0] for b in range(B)]
    max_nb = max(len(i) for i in idx)
    NA = max(256, ((max_nb + 127) // 128) * 128)
    QC = max(128, ((max_nb + 63) // 64) * 64)

    if (NA, QC) not in _CACHE:
        _CACHE[(NA, QC)] = build(NA, QC)
    nc = _CACHE[(NA, QC)]

    wmaps = _prep_weights(*args)
    in_maps = _make_in_maps(hidden_states, idx, NA, wmaps)

    res = run_bass_kernel_spmd(nc, in_maps, core_ids=list(range(8)))

    out = np.zeros((B, S, D), np.float32)
    for b in range(B):
        nb = len(idx[b])
        oT = (res.results[2 * b]["out"].astype(np.float32)
              + res.results[2 * b + 1]["out"].astype(np.float32))
        out[b, idx[b]] = oT.T[:nb] + bo_f[None, :]
    return out



# revision 32
# speedup vs baseline: 1.0410x; 1.0005x over previous
"""AnchorAttention Trainium2 kernel (8 NeuronCores, SPMD, no collectives).

Math (per batch): gather anchor rows of hidden_states, LayerNorm, QKV
projections, dense attention among anchors only, out-projection, scatter
back (non-anchor rows of the output are zero; keys are anchors only).

Sharding: core c handles batch c//2 and HEAD GROUP c%2 (4 of 8 heads).
Both cores of a pair see the same gathered anchor tokens; each computes
q/k/v and attention for its 4 heads over ALL anchors, then a partial
out-projection (sum over its heads). The host adds the two partials
(+ output bias) — out-projection is linear in heads, so no collective
is needed.

Device layout (contraction dims on partitions):
  zT   per 512-token chunk: (128, 6, 512)  z = (x-mu)*rstd, d on partitions
  qT   (128, 4, NA)  per head 128 rows: 96 hd + row96 == 1.0 (mask helper)
  kT   (128, 4, NA)  per head 128 rows: 96 hd + row96 == key-pad mask
  v    (128, T, 4, 128) plain layout: 96 head dims + ones col + zero pad
  scores^T (tk, tq) per (head, 2-tile key group, query-chunk); one
  1024-wide exp per group; probs = exp(scale*s)
  avT  (128, NQH) accumulated over tk; row 96 = softmax denominator
  outT (768, NA) = sum_h Wo_h^T @ (avT_h / denom_h)   [bias added on host]

LayerNorm's affine (ln_g, ln_b) is folded into the weights on the host:
W~ = W * g, bias~ = W @ b + bias.
"""

import numpy as np
import ml_dtypes

import concourse.bass as bass
import concourse.mybir as mybir
import concourse.tile as tile
from concourse import bacc
from concourse.bass_utils import run_bass_kernel_spmd

BF16 = ml_dtypes.bfloat16
F32 = mybir.dt.float32
BF = mybir.dt.bfloat16

B, S, D, H, HD = 4, 2048, 768, 8, 96
HL = H // 2           # heads per core
J = D // 128          # contraction blocks
EPS = 1e-5
SCALE = 1.0 / np.sqrt(HD)
MASK_NEG = -800.0     # exp(SCALE*(qk+MASK_NEG)) ~ 4e-36 (and keeps the
                      # Schraudolph int16 path positive / unsaturated)
LOG2E = 1.4426950408889634
SCH_A = SCALE * 128.0 * LOG2E        # bf16-bits = round(score*SCH_A + SCH_B)
SCH_B = (127.0 - 0.06) * 128.0       # c=0.06 minimizes rms rel err (~1.8%)


def _chunks(total, step):
    out = []
    c = 0
    while c < total:
        out.append((c, min(step, total - c)))
        c += step
    return out


def build(NA, QC):
    """Build the per-core Bacc graph for padded anchor count NA."""
    assert NA % 128 == 0 and QC % 64 == 0 and QC <= NA
    T = NA // 128
    # attention query units (<= 512 wide); the ragged tail chunk stays last:
    # with the rolling carry its iterations overlap the previous chunk's
    # out-projection, and the final tail/out-proj ops are narrow (fast).
    QSPLIT = _chunks(QC, 512)
    # token chunks: a short first chunk lets the K/Q projections start
    # before LayerNorm (the prologue critical path) finishes later tiles.
    if NA > 256:
        CH = [(0, 256)] + [(c0 + 256, cw) for (c0, cw) in _chunks(NA - 256, 512)]
    else:
        CH = [(0, NA)]

    nc = bacc.Bacc("TRN2", target_bir_lowering=False, debug=False, num_devices=8)

    # x arrives host-packed as [128, T*D]: partition p holds tokens
    # p, 128+p, ... so each DMA line is T*1536 contiguous bytes per
    # partition (full HBM bandwidth; the [NA, D] layout only manages
    # 1536-byte lines).
    x_ext = nc.dram_tensor("x", [128, T * D], BF, kind="ExternalInput").ap()
    wq_ext = nc.dram_tensor("wq", [128, J * HL * 128], BF, kind="ExternalInput").ap()
    wk_ext = nc.dram_tensor("wk", [128, J * HL * 128], BF, kind="ExternalInput").ap()
    wv_ext = nc.dram_tensor("wv", [128, J * HL * 96], BF, kind="ExternalInput").ap()
    wo_ext = nc.dram_tensor("wo", [128, HL * D], BF, kind="ExternalInput").ap()
    bq_ext = nc.dram_tensor("bq", [128, HL], F32, kind="ExternalInput").ap()
    bk_ext = nc.dram_tensor("bk", [128, HL], F32, kind="ExternalInput").ap()
    bv_ext = nc.dram_tensor("bv", [HL * 96], F32, kind="ExternalInput").ap()
    km_ext = nc.dram_tensor("km", [1, NA], BF, kind="ExternalInput").ap()
    # output staged as [128, nq, J, 512]: one 6KB-per-partition-contiguous
    # DMA per query chunk (the [D, QC] layout only gives 1KB DMA lines).
    NQS = len(QSPLIT)
    out_ext = nc.dram_tensor(
        "out", [128, NQS * J * 512], BF, kind="ExternalOutput").ap()
    out_v = out_ext.rearrange("p (q m c) -> p q m c", q=NQS, m=J)

    with tile.TileContext(nc) as tc:
        with (
            tc.tile_pool(name="singles", bufs=1) as singles,
            tc.tile_pool(name="work", bufs=5) as work,
            tc.tile_pool(name="probs", bufs=6) as probs_pool,
        ):
            # ---- x first (LN needs it immediately; queues are FIFO so
            # anything emitted before it would delay the whole prologue).
            # Two halves so LN can start after the first 4 tiles land.
            x_all = singles.tile([128, T, D], BF)
            x_v = x_ext.rearrange("p (t d) -> p t d", t=T)
            nc.sync.dma_start(out=x_all[:, 0:1, :], in_=x_v[:, 0:1, :])
            for (i0, iw) in _chunks(T - 1, 2):
                nc.sync.dma_start(
                    out=x_all[:, 1 + i0:1 + i0 + iw, :],
                    in_=x_v[:, 1 + i0:1 + i0 + iw, :])
            x_tiles = [x_all[:, i, :] for i in range(T)]

            # ---- weights / constants into SBUF (one DMA per tensor, issued
            # in consumption order: K first, Wo last).
            wq_sb = singles.tile([128, J, HL * 128], BF)
            wk_sb = singles.tile([128, J, HL * 128], BF)
            wv_sb = singles.tile([128, J, HL * 96], BF)
            wo_sb = singles.tile([128, HL, D], BF)
            nc.sync.dma_start(
                out=wk_sb, in_=wk_ext.rearrange("p (j e) -> p j e", j=J))
            nc.sync.dma_start(
                out=wq_sb, in_=wq_ext.rearrange("p (j e) -> p j e", j=J))
            nc.sync.dma_start(
                out=wv_sb, in_=wv_ext.rearrange("p (j e) -> p j e", j=J))
            nc.sync.dma_start(out=wo_sb, in_=wo_ext)
            bq_sb = singles.tile([128, HL], F32)
            nc.gpsimd.dma_start(out=bq_sb, in_=bq_ext)
            bk_sb = singles.tile([128, HL], F32)
            nc.gpsimd.dma_start(out=bk_sb, in_=bk_ext)
            bv_sb = singles.tile([128, HL * 96], F32)
            bv_bcast = bass.AP(
                tensor=bv_ext.tensor, offset=bv_ext.offset,
                ap=[[0, 128], [1, HL * 96]],
            )
            nc.gpsimd.dma_start(out=bv_sb, in_=bv_bcast)

            ones96 = singles.tile([1, 96], BF)
            nc.vector.memset(ones96, 1.0)
            eps_sb = singles.tile([128, 1], F32)
            nc.vector.memset(eps_sb, EPS)
            ident = singles.tile([128, 128], BF)
            from concourse.masks import make_identity
            make_identity(nc, ident)

            # HAM warmup: ~45 small matmuls on the identity while the x DMA
            # and LayerNorm are still in flight. The PE clock-gate needs
            # ~3.4us of sustained matmul activity to release 2.4 GHz (and
            # transpose-mode doesn't count) — without this the first real
            # projection matmuls run at 1.2 GHz.
            with tc.tile_pool(name="ps_w", bufs=1, space="PSUM") as ps_w:
                warm_ps = ps_w.tile([128, 128], F32)
                for _ in range(45):
                    nc.tensor.matmul(
                        warm_ps, lhsT=ident, rhs=ident, start=True, stop=True)
            zT = [singles.tile([128, J, cw], BF, name=f"zT{c}")
                  for c, (c0, cw) in enumerate(CH)]

            def zt_slice(j, c0, cw):
                ci = max(i for i, (cc0, _) in enumerate(CH) if cc0 <= c0)
                off = c0 - CH[ci][0]
                assert off + cw <= CH[ci][1]
                return zT[ci][:, j, off:off + cw]

            kT = singles.tile([128, HL, NA], BF)
            qT = singles.tile([128, HL, QC], BF)
            v_sb = singles.tile([128, T, HL, 128], BF)
            # per-head tiles so the out-projection's per-head matmuls only
            # depend on their own head's tail (not the whole buffer)
            avn = [singles.tile([128, QC], BF, name=f"avn{h}")
                   for h in range(HL)]
            for h in range(HL):
                nc.gpsimd.memset(avn[h][96:128, :], 0.0)

            # v columns: 0..95 head dims, 96 ones (denominator), 97.. zero
            # (padding to 128 weights keeps FWL on for the av matmuls)
            nc.vector.memset(v_sb[:, :, :, 96:97], 1.0)
            nc.gpsimd.memset(v_sb[:, :, :, 97:128], 0.0)

            with (
                tc.tile_pool(name="ps_proj", bufs=2, space="PSUM") as ps_proj,
                tc.tile_pool(name="ps_t", bufs=3, space="PSUM") as ps_t,
            ):
                # Pipeline per 512-token chunk: LN/z -> transpose (on the
                # otherwise-idle PE) -> K/Q/V projections for that chunk.
                for ci, (c0, cw) in enumerate(CH):
                    tlo, thi = c0 // 128, (c0 + cw) // 128
                    for i in range(tlo, thi):
                        x_i = x_tiles[i]
                        x_g = x_i.rearrange("p (n f) -> p n f", f=384)
                        stats = work.tile([128, 2, 6], F32, tag="stats")
                        for g in range(2):
                            nc.vector.bn_stats(out=stats[:, g, :], in_=x_g[:, g, :])
                        mv = work.tile([128, 2], F32, tag="mv")
                        nc.vector.bn_aggr(out=mv, in_=stats)
                        sd = work.tile([128, 1], F32, tag="sd")
                        nc.scalar.activation(
                            out=sd, in_=mv[:, 1:2],
                            func=mybir.ActivationFunctionType.Sqrt,
                            bias=eps_sb, scale=1.0,
                        )
                        rstd = work.tile([128, 1], F32, tag="rstd")
                        nc.vector.reciprocal(out=rstd, in_=sd)
                        z_i = work.tile([128, D], BF, tag="z")
                        nc.vector.tensor_scalar(
                            out=z_i, in0=x_i,
                            scalar1=mv[:, 0:1], scalar2=rstd,
                            op0=mybir.AluOpType.subtract, op1=mybir.AluOpType.mult,
                        )
                        ioff = (i - tlo) * 128
                        tp = ps_t.tile([128, J, 128], BF, tag="tp")
                        for j in range(J):
                            nc.tensor.transpose(
                                tp[:, j, :], z_i[:, j * 128:(j + 1) * 128],
                                ident)
                        nc.scalar.activation(
                            out=zT[ci][:, :, ioff:ioff + 128], in_=tp,
                            func=mybir.ActivationFunctionType.Copy,
                        )

                    # K / Q projections for this chunk (local heads).
                    # Q only covers [0, QC) — queries past the last anchor
                    # are never read.
                    qw_c = min(cw, max(0, QC - c0))
                    for (name, w_sb, b_sb, dst, ncols) in (
                        ("k", wk_sb, bk_sb, kT, cw),
                        ("q", wq_sb, bq_sb, qT, qw_c),
                    ):
                        if ncols == 0:
                            continue
                        for m in range(HL):
                            ps = ps_proj.tile([128, ncols], F32, tag="proj")
                            for j in range(J):
                                nc.tensor.matmul(
                                    ps,
                                    lhsT=w_sb[:, j, m * 128:(m + 1) * 128],
                                    rhs=zT[ci][:, j, :ncols],
                                    start=(j == 0), stop=(j == J - 1),
                                )
                            nc.vector.tensor_scalar_add(
                                out=dst[:, m, c0:c0 + ncols], in0=ps,
                                scalar1=b_sb[:, m:m + 1],
                            )
                    # V projection for this chunk's token tiles (all 4 local
                    # heads in one N=384 matmul per contraction block)
                    for i in range(tlo, thi):
                        ps = ps_proj.tile([128, HL * 96], F32, tag="proj")
                        for j in range(J):
                            nc.tensor.matmul(
                                ps,
                                lhsT=zt_slice(j, i * 128, 128),
                                rhs=wv_sb[:, j, :],
                                start=(j == 0), stop=(j == J - 1),
                            )
                        nc.vector.tensor_tensor(
                            out=v_sb[:, i, :, 0:96],
                            in0=ps.rearrange("p (h c) -> p h c", c=96),
                            in1=bv_sb.rearrange("p (h c) -> p h c", c=96),
                            op=mybir.AluOpType.add,
                        )

                # overwrite kT row 96 of every head with the key-pad mask row
                km_bcast = bass.AP(
                    tensor=km_ext.tensor, offset=km_ext.offset,
                    ap=[[0, 1], [0, HL], [1, NA]],
                )
                nc.gpsimd.dma_start(out=kT[96:97, :, :], in_=km_bcast)

            # ---- attention + out-projection, fused per query chunk.
            # Per (query-chunk, head): scores for 2 key-tiles land in a
            # 2-bank PSUM group; ONE 1024-wide exp per group (halves the
            # per-instruction overhead on ScalarE — the phase bottleneck).
            # av matmuls for group g are emitted right after exp g, so the
            # in-order TensorE executes them while exp g+1 runs. After all
            # heads of a chunk, that chunk's out-projection is emitted; its
            # matmuls + output DMA overlap the next chunk's attention.
            # PSUM budget: scores 2x2 + av 2 + out-proj 2 = 8 banks.
            TKG = _chunks(T, 2)

            with (
                tc.tile_pool(name="ps_s", bufs=2, space="PSUM") as ps_s,
                tc.tile_pool(name="ps_av", bufs=2, space="PSUM") as ps_av,
                tc.tile_pool(name="ps_o", bufs=2, space="PSUM") as ps_o,
            ):
                def emit_av(h, grp, av_ps, qw):
                    t0, tn, pb = grp
                    for gi in range(tn):
                        tk = t0 + gi
                        nc.tensor.matmul(
                            av_ps[:, :qw],
                            lhsT=v_sb[:, tk, h, :],
                            rhs=pb[:, gi, :qw],
                            start=(tk == 0), stop=(tk == T - 1),
                            skip_group_check=True,
                        )

                def tail(h, q0, qw, av_ps):
                    # normalize: avn = avT[0:96] * (1 / avT[96]) broadcast.
                    # (recip_approx_fast is a bitwise-seed custom op — needs
                    # its input in SBUF, so copy the denominator row first.)
                    d_sb = work.tile([1, qw], F32, tag="dsb")
                    nc.vector.tensor_copy(out=d_sb, in_=av_ps[96:97, :qw])
                    rec32 = work.tile([1, qw], F32, tag="rec32")
                    nc.vector.reciprocal_approx_fast(out=rec32, in_=d_sb)
                    recip_bf = work.tile([1, qw], BF, tag="recipbf")
                    nc.vector.tensor_copy(out=recip_bf, in_=rec32)
                    bc_sb = work.tile([96, qw], BF, tag="bc")
                    nc.gpsimd.partition_broadcast(out_ap=bc_sb, in_ap=recip_bf)
                    nc.vector.tensor_tensor(
                        out=avn[h][0:96, q0:q0 + qw],
                        in0=av_ps[0:96, :qw], in1=bc_sb,
                        op=mybir.AluOpType.mult,
                    )

                # exp groups handled by DVE (Schraudolph bf16 bit-trick,
                # ~1.8% rms) instead of ScalarE's exact LUT exp: every 3rd
                # group starting at 1 (so ~1/3 of the exp stream moves off
                # the ScalarE bottleneck; error contribution ~1%).
                # never the last group: its exp gates the carry flush (av +
                # tail) and the DVE queue is backlogged at that point
                ngrp = len(TKG)
                if ngrp >= 5:
                    dve_groups = {g for g in range(1, ngrp - 1, 2)}
                elif ngrp >= 3:
                    dve_groups = {1}
                else:
                    dve_groups = set()

                o_all = singles.tile([128, NQS, J, 512], BF)
                qci_of = {q0: i for i, (q0, _) in enumerate(QSPLIT)}

                def emit_outproj(q0, qw):
                    # partial out projection for one query chunk (sum over
                    # local heads; host adds the pair partials + bias)
                    qci = qci_of[q0]
                    for m in range(J):
                        o_ps = ps_o.tile([128, 512], F32, tag="o")
                        for hh in range(HL):
                            nc.tensor.matmul(
                                o_ps[:, :qw],
                                lhsT=wo_sb[:, hh, m * 128:(m + 1) * 128],
                                rhs=avn[hh][:, q0:q0 + qw],
                                start=(hh == 0), stop=(hh == HL - 1),
                            )
                        # ScalarE evacuates (DVE is loaded with Schraudolph
                        # exps + tails in this phase)
                        nc.scalar.activation(
                            out=o_all[:, qci, m, :qw], in_=o_ps[:, :qw],
                            func=mybir.ActivationFunctionType.Copy,
                        )
                    eng = nc.sync if (qci % 2 == 0) else nc.gpsimd
                    eng.dma_start(
                        out=out_v[:, qci, :, :], in_=o_all[:, qci, :, :])

                # Rolling software pipeline across (query-chunk, head)
                # iterations: iteration i's last av group + tail (and, after
                # a chunk's final head, that chunk's out-projection) are
                # emitted inside iteration i+1, after its first exp — so the
                # in-order TensorE always has the next iteration's scores to
                # chew on while ScalarE/VectorE finish iteration i.
                iters = [(q0, qw, h) for (q0, qw) in QSPLIT for h in range(HL)]
                carry = None          # (h, pend_grp, av_ps, q0, qw, oproj)
                for (q0, qw, h) in iters:
                    av_ps = ps_av.tile([128, 512], F32, tag="av")
                    pend = None
                    for g, (t0, tn) in enumerate(TKG):
                        s_ps = ps_s.tile([128, 2, 512], F32, tag="s")
                        for gi in range(tn):
                            tk = t0 + gi
                            nc.tensor.matmul(
                                s_ps[:, gi, :qw],
                                lhsT=kT[:, h, tk * 128:(tk + 1) * 128],
                                rhs=qT[:, h, q0:q0 + qw],
                                start=True, stop=True,
                            )
                        pb = probs_pool.tile([128, 2, 512], BF, tag="p")
                        if g in dve_groups:
                            nc.vector.tensor_scalar(
                                out=pb.bitcast(mybir.dt.int16)[:, :tn, :qw],
                                in0=s_ps[:, :tn, :qw],
                                scalar1=float(SCH_A), scalar2=float(SCH_B),
                                op0=mybir.AluOpType.mult,
                                op1=mybir.AluOpType.add,
                            )
                        else:
                            nc.scalar.activation(
                                out=pb[:, :tn, :qw], in_=s_ps[:, :tn, :qw],
                                func=mybir.ActivationFunctionType.Exp,
                                scale=float(SCALE),
                            )
                        if g == 0 and carry is not None:
                            ch_, cpend, cav, cq0, cqw, coproj = carry
                            emit_av(ch_, cpend, cav, cqw)
                            tail(ch_, cq0, cqw, cav)
                            if coproj:
                                emit_outproj(cq0, cqw)
                            carry = None
                        if pend is not None:
                            emit_av(h, pend, av_ps, qw)
                        pend = (t0, tn, pb)
                    carry = (h, pend, av_ps, q0, qw, h == HL - 1)
                ch_, cpend, cav, cq0, cqw, coproj = carry
                emit_av(ch_, cpend, cav, cqw)
                tail(ch_, cq0, cqw, cav)
                if coproj:
                    emit_outproj(cq0, cqw)

    nc.compile()
    return nc


_CACHE = {}


def _prep_weights(ln_g, ln_b, Wq, bq, Wk, bk, Wv, bv, Wo, bo):
    """Per-head-group device weight layouts. Returns [group0, group1]."""
    def pad_head_T(W, hg):
        # (W * g).T for heads of the group, padded 96 -> 128 cols, then
        # SBUF layout (128, J, HL*128): [p, j, e] = WT[j*128+p, e]
        WT = (W * ln_g[None, :]).T.astype(np.float32)
        WT = WT.reshape(D, H, 96)[:, hg * HL:(hg + 1) * HL, :]
        Wp = np.zeros((D, HL, 128), np.float32)
        Wp[:, :, :96] = WT
        Wp = Wp.reshape(J, 128, HL * 128).transpose(1, 0, 2)
        return np.ascontiguousarray(Wp.reshape(128, J * HL * 128)).astype(BF16)

    def plain_T(W, hg):
        WT = (W * ln_g[None, :]).T.astype(np.float32)
        WT = WT.reshape(D, H, 96)[:, hg * HL:(hg + 1) * HL, :].reshape(D, HL * 96)
        Wp = WT.reshape(J, 128, HL * 96).transpose(1, 0, 2)
        return np.ascontiguousarray(Wp.reshape(128, J * HL * 96)).astype(BF16)

    def pad_bias(bb, hg, ones_row):
        bp = np.zeros((HL, 128), np.float32)
        bp[:, :96] = bb.reshape(H, 96)[hg * HL:(hg + 1) * HL]
        if ones_row:
            bp[:, 96] = 1.0
        return np.ascontiguousarray(bp.T).astype(np.float32)  # (128, HL)

    def pad_wo(hg):
        w = np.zeros((128, HL, D), np.float32)
        w[:96] = Wo.T.reshape(H, 96, D)[hg * HL:(hg + 1) * HL].transpose(1, 0, 2)
        return np.ascontiguousarray(w.reshape(128, HL * D)).astype(BF16)

    bbq = Wq @ ln_b + bq
    bbk = Wk @ ln_b + bk
    bbv = Wv @ ln_b + bv
    return [{
        "wq": pad_head_T(Wq, hg),
        "wk": pad_head_T(Wk, hg),
        "wv": plain_T(Wv, hg),
        "wo": pad_wo(hg),
        "bq": pad_bias(bbq, hg, True),
        "bk": pad_bias(bbk, hg, False),
        "bv": np.ascontiguousarray(
            bbv.reshape(H, 96)[hg * HL:(hg + 1) * HL].reshape(-1)
        ).astype(np.float32),
    } for hg in range(2)]


def _make_in_maps(hidden_states, idx, NA, wmaps):
    in_maps = []
    for c in range(8):
        b, hg = c // 2, c % 2
        nb = len(idx[b])
        xg = np.zeros((NA, D), np.float32)
        xg[:nb] = hidden_states[b][idx[b]]
        # pack [NA, D] -> [128, T*D]: partition p gets tokens p, 128+p, ...
        T = NA // 128
        xp = np.ascontiguousarray(
            xg.reshape(T, 128, D).transpose(1, 0, 2).reshape(128, T * D))
        km = np.zeros((NA,), np.float32)
        km[nb:] = MASK_NEG
        in_maps.append({
            "x": xp.astype(BF16),
            "km": km.reshape(1, NA).astype(BF16),
            **wmaps[hg],
        })
    return in_maps


def kernel(hidden_states, anchor_mask, ln_g, ln_b,
           Wq, bq, Wk, bk, Wv, bv, Wo, bo):
    hidden_states = np.asarray(hidden_states, dtype=np.float32)
    anchor_mask = np.asarray(anchor_mask).astype(bool)
    args = [np.asarray(a, dtype=np.float32)
            for a in (ln_g, ln_b, Wq, bq, Wk, bk, Wv, bv, Wo, bo)]
    bo_f = args[-1]

    idx = [np.nonzero(anchor_mask[b])[0] for b in range(B)]
    max_nb = max(len(i) for i in idx)
    NA = max(256, ((max_nb + 127) // 128) * 128)
    QC = max(128, ((max_nb + 63) // 64) * 64)

    if (NA, QC) not in _CACHE:
        _CACHE[(NA, QC)] = build(NA, QC)
    nc = _CACHE[(NA, QC)]

    wmaps = _prep_weights(*args)
    in_maps = _make_in_maps(hidden_states, idx, NA, wmaps)

    res = run_bass_kernel_spmd(nc, in_maps, core_ids=list(range(8)))

    qsplit = _chunks(QC, 512)
    nqs = len(qsplit)

    def unpack(buf):
        # [128, nqs, J, 512] staged layout -> outT [D, QC]
        b4 = buf.reshape(128, nqs, J, 512).astype(np.float32)
        oT = np.zeros((J, 128, QC), np.float32)
        for qci, (q0, qw) in enumerate(qsplit):
            oT[:, :, q0:q0 + qw] = b4[:, qci, :, :qw].transpose(1, 0, 2)
        return oT.reshape(D, QC)

    out = np.zeros((B, S, D), np.float32)
    for b in range(B):
        nb = len(idx[b])
        oT = (unpack(res.results[2 * b]["out"])
              + unpack(res.results[2 * b + 1]["out"]))
        out[b, idx[b]] = oT.T[:nb] + bo_f[None, :]
    return out

